# revision 1
# baseline (speedup 1.0000x reference)
"""Trainium2 Bass kernel for nn_BiDenseConv2d (binarized 3x3 conv + sync-BN + channel bypass).

Shapes (hardcoded): x [8, 48, 224, 224] f32 -> out [8, 64, 224, 224] f32.

Sharding: data-parallel over batch, 1 image per NeuronCore (8 cores); BN batch
stats all-reduced across cores ([64,2] f32 collective); weights replicated.

Per-core pipeline:
  1. binarize: sign(sin(2pi(x-eps)/tau)) == +1 iff t - rint(t) >= 0 where
     t=(x-eps)/tau; rint via the fp32 magic constant (1.5*2^23), split across
     ACT (add) and DVE (sub, compare) -> {-0.5,+0.5} in fp8e4. Runs in a
     seg-major [128p] layout (partition = 16*seg + group) fed by a
     host-prearranged copy of x so every DMA is 128 partitions wide.
  2. conv: 9-tap shift-matmul, kh-pairs stacked to K=96 via a one-row-shifted
     image copy on partitions 48..95; two output blocks run concurrently via PE
     column tiling (0,0)/(0,64). fp8 +-0.5 acts x +-1 weights accumulate exact
     half-integer sums in PSUM f32; evicted to fp16 (exact).
  3. BN: sums/sumsq via accum_out on the eviction ops; AllReduce; affine
     k = gamma*s'*rsqrt(s'^2 var + eps), c = beta - mu k with s' = 2 mean|w|.
  4. bypass: identity channels stream from HBM (channel-major x input); the 16
     merge-mean channels are 3-channel group means computed in the seg-major
     layout (GPSIMD) and merged into the bypass buffer by DMA.

Conv input channel order is a permutation (slot 16c+g <-> channel 15c+g, g<15;
45+c for g=15) folded into the weights host-side.
"""
import sys
import numpy as np

sys.path.insert(0, '/opt/trn_rl_repo')

B, CIN, COUT, H, W = 8, 48, 64, 224, 224
NCORES = 8
SEGS, SEGR = 8, 28          # 8 row-segments of 28 rows
SEGQ = SEGR * W             # 6272
HSEGQ = SEGQ // 2           # 3136
NBANK = 56                  # bank b covers image rows 4b..4b+3
NEG = 14                    # eviction groups of 4 banks
PW = 226
BN_EPS = 1e-5
MAGIC = 12582912.0          # 1.5 * 2**23: fp32 round-to-int magic

_cache = {}


class _StopBuild(Exception):
    pass

# slot permutation: conv channel-slot 16c+g holds channel 15c+g (g<15), 45+c (g=15)
SLOT_TO_CH = np.zeros(48, np.int64)
for _c in range(3):
    for _g in range(16):
        SLOT_TO_CH[16 * _c + _g] = (45 + _c) if _g == 15 else (15 * _c + _g)


def _build(general_affine: bool, fake_kc: bool = False, prep_probe: bool = False):
    from concourse import bacc, tile, mybir
    mt = mybir.dt
    AO = mybir.AluOpType
    AF = mybir.ActivationFunctionType

    nc = bacc.Bacc("TRN2", target_bir_lowering=False, debug=False,
                   num_devices=NCORES)

    xdev_d = nc.dram_tensor("xdev", [128, 3, SEGQ], mt.float32, kind="ExternalInput")
    xch_d = nc.dram_tensor("xch", [CIN, H * W], mt.float32, kind="ExternalInput")
    wp_d = nc.dram_tensor("wp", [3, 96, 64], mt.float8e4, kind="ExternalInput")
    ws_d = nc.dram_tensor("ws", [3, 48, 64], mt.float8e4, kind="ExternalInput")
    cst_d = nc.dram_tensor("cst", [64, 4], mt.float32, kind="ExternalInput")
    coef_d = nc.dram_tensor("coef", [128, 8], mt.float32, kind="ExternalInput")
    out_d = nc.dram_tensor("out", [2, COUT, NBANK, 448], mt.float32,
                           kind="ExternalOutput")

    xv_blk = xch_d.ap().rearrange("c (s j p w) -> c s j p w", s=SEGS, j=7, p=2)

    try:
        with tile.TileContext(nc) as tc:
            with tc.tile_pool(name="main", bufs=1) as P, \
                 tc.tile_pool(name="psum", bufs=2, space="PSUM") as PS, \
                 tc.tile_pool(name="dram", bufs=1, space="DRAM") as D:

                # ---- constants ----
                wp = P.tile([96, 3, 64], mt.float8e4)
                ws = P.tile([48, 3, 64], mt.float8e4)
                for kw in range(3):
                    nc.sync.dma_start(wp[:, kw, :], wp_d.ap()[kw])
                    nc.sync.dma_start(ws[:, kw, :], ws_d.ap()[kw])
                cst = P.tile([64, 4], mt.float32)
                nc.sync.dma_start(cst[:], cst_d.ap())
                coef = P.tile([128, 8], mt.float32)
                if general_affine:
                    nc.sync.dma_start(coef[:], coef_d.ap())
                magic_t = P.tile([128, 1], mt.float32)
                nc.vector.memset(magic_t[:], MAGIC)

                # ---- persistent tiles ----
                xa2f = P.tile([96, PW, PW], mt.float8e4)
                bm = P.tile([128, 2, HSEGQ], mt.float32)
                y = P.tile([128, NBANK, 448], mt.float16)
                sums = P.tile([128, NEG], mt.float32)
                sqs = P.tile([128, NEG], mt.float32)

                # zero borders (compute partition bases must be 0/32/64/96, so
                # these span [0:96]; interior rows rewritten by scatter/B-copy).
                # The strided column borders are DVE copies from a zero tile: a
                # strided memset is engine-agnostic and can land on GPSIMD, where
                # it costs ~95us.
                nc.vector.memset(xa2f[0:96, 0, :], 0.0)
                nc.vector.memset(xa2f[0:96, 224:226, :], 0.0)
                zrow = P.tile([96, 226], mt.float8e4)
                nc.vector.memset(zrow[:], 0.0)
                nc.vector.tensor_copy(xa2f[0:96, :, 0], zrow[:])
                nc.vector.tensor_copy(xa2f[0:96, :, 225], zrow[:])

                # ---- prep: load, binarize, scatter ----
                for c in range(3):
                    for hf in range(2):
                        x1b = P.tile([128, HSEGQ], mt.float32, tag="x1", bufs=4,
                                     name=f"x1b_{c}_{hf}")
                        nc.sync.dma_start(
                            x1b[:], xdev_d.ap()[:, c, hf * HSEGQ:(hf + 1) * HSEGQ])
                        if general_affine:
                            nc.vector.tensor_scalar(
                                x1b[:], x1b[:], coef[:, c:c + 1], coef[:, 3 + c:4 + c],
                                AO.mult, AO.add)
                        # bypass group-sums on GPSIMD (least-busy engine)
                        if c == 0:
                            nc.gpsimd.tensor_copy(bm[:, hf, :], x1b[:])
                        else:
                            nc.gpsimd.tensor_tensor(bm[:, hf, :], bm[:, hf, :],
                                                    x1b[:], AO.add)
                        # rint(t) = (t + MAGIC) - MAGIC, then sign = (t >= rint)
                        m1 = P.tile([128, HSEGQ], mt.float32, tag="t2ob", bufs=2,
                                    name=f"m1_{c}_{hf}")
                        nc.vector.tensor_scalar(m1[:], x1b[:], MAGIC, MAGIC,
                                                AO.add, AO.subtract)
                        t2b = P.tile([128, HSEGQ], mt.bfloat16, tag="t2ob", bufs=2,
                                     name=f"t2b_{c}_{hf}")
                        nc.vector.tensor_tensor(t2b[:], x1b[:], m1[:], AO.is_ge)
                        xa1b = P.tile([128, HSEGQ], mt.float8e4, tag="xa1bp", bufs=2,
                                      name=f"xa1b_{c}_{hf}")
                        nc.vector.tensor_scalar(xa1b[:], t2b[:], 0.5, None,
                                                AO.subtract)
                        # scatter to conv layout: one 16-partition DMA per segment
                        for s in range(SEGS):
                            r0 = 1 + SEGR * s + 14 * hf
                            nc.scalar.dma_start(
                                xa2f[16 * c:16 * c + 16, r0:r0 + 14, 1:225],
                                xa1b[16 * s:16 * s + 16, :].rearrange(
                                    "p (r w) -> p r w", r=14))

                # B half: one-row-shifted copy of A, per segment
                for s in range(SEGS):
                    nc.scalar.dma_start(xa2f[48:96, SEGR * s:SEGR * s + SEGR, :],
                                        xa2f[0:48, SEGR * s + 1:SEGR * s + SEGR + 1, :])

                bmf = bm[:].rearrange("p h q -> p (h q)")
                nc.gpsimd.tensor_scalar(bmf, bmf, 1.0 / 3.0, None, AO.mult)

                # ---- conv ----
                if prep_probe:
                    probe_d = nc.dram_tensor("probe", [96, PW * PW], mt.float8e4,
                                             kind="ExternalOutput")
                    nc.sync.dma_start(probe_d.ap(),
                                      xa2f[:].rearrange("p a b -> p (a b)"))

                for b4 in range(NEG if not prep_probe else 0):
                    ps4 = PS.tile([128, 4, 512], mt.float32, tag="ps", bufs=2,
                                  name=f"ps4_{b4}")
                    for k in range(4):
                        b = 4 * b4 + k
                        for ci, (pb, tp) in enumerate(((0, (0, 0)), (64, (0, 64)))):
                            h0 = 4 * b + 2 * ci
                            for kw in range(3):
                                nc.tensor.matmul(
                                    ps4[pb:pb + 64, k, 0:448],
                                    wp[:, kw, :],
                                    xa2f[0:96, h0:h0 + 2, kw:kw + 224],
                                    start=(kw == 0), stop=False, tile_position=tp)
                            for kw in range(3):
                                nc.tensor.matmul(
                                    ps4[pb:pb + 64, k, 0:448],
                                    ws[:, kw, :],
                                    xa2f[0:48, h0 + 2:h0 + 4, kw:kw + 224],
                                    start=False, stop=(kw == 2), tile_position=tp)
                    nc.vector.tensor_scalar(
                        y[:, 4 * b4:4 * b4 + 4, :], ps4[:, :, 0:448], 1.0, None,
                        AO.mult, AO.add, accum_out=sums[:, b4:b4 + 1])
                    nc.scalar.activation(ps4[:, :, 0:448], ps4[:, :, 0:448],
                                         AF.Square, accum_out=sqs[:, b4:b4 + 1])

                bmflat = bm[:].rearrange("p h q -> p (h q)")

                def load_bpb(s, tag="x1"):
                    bpb = P.tile([128, 7, 448], mt.float32, tag=tag, bufs=4,
                                 name=f"bpb_{s}")
                    nc.sync.dma_start(bpb[0:48, :, :], xv_blk[:, s, :, 0, :])
                    nc.sync.dma_start(bpb[64:112, :, :], xv_blk[:, s, :, 1, :])
                    bmsrc = bmflat[16 * s:16 * s + 16, :].rearrange(
                        "p (j e) -> p j e", j=7)
                    nc.sync.dma_start(bpb[48:64, :, :], bmsrc[:, :, 0:448])
                    nc.sync.dma_start(bpb[112:128, :, :], bmsrc[:, :, 448:896])
                    return bpb

                bpb_tiles = {s: load_bpb(s) for s in range(4)}

                # ---- stats + collective + BN affine ----
                if prep_probe:
                    raise _StopBuild()
                kc = P.tile([128, 2], mt.float32)
                if fake_kc:
                    nc.vector.memset(kc[:], 1.0)   # timing experiment only
                else:
                    ssum = P.tile([128, 1], mt.float32)
                    ssq = P.tile([128, 1], mt.float32)
                    nc.vector.reduce_sum(ssum[:], sums[:], axis=mybir.AxisListType.X)
                    nc.vector.reduce_sum(ssq[:], sqs[:], axis=mybir.AxisListType.X)
                    toph = P.tile([64, 2], mt.float32)
                    nc.sync.dma_start(toph[:, 0:1], ssum[64:128, :])
                    nc.sync.dma_start(toph[:, 1:2], ssq[64:128, :])
                    cb = P.tile([64, 2], mt.float32)
                    nc.vector.tensor_tensor(cb[:, 0:1], ssum[0:64, :], toph[:, 0:1],
                                            AO.add)
                    nc.vector.tensor_tensor(cb[:, 1:2], ssq[0:64, :], toph[:, 1:2],
                                            AO.add)
                    nc.vector.tensor_scalar(cb[:], cb[:], 1.0 / float(B * H * W),
                                            None, AO.mult)
                    cbin = D.tile([64, 2], mt.float32)
                    cbout = D.tile([NCORES, 64, 2], mt.float32)
                    nc.sync.dma_start(cbin[:], cb[:])
                    nc.gpsimd.collective_compute(
                        "AllGather", AO.bypass,
                        replica_groups=[list(range(NCORES))],
                        ins=[cbin.opt()], outs=[cbout.opt()])
                    # gather to SBUF as [64, 2, 8] and reduce the replica dim
                    gath = P.tile([64, 2, NCORES], mt.float32)
                    nc.sync.dma_start(
                        gath[:],
                        cbout[:].rearrange("g p q -> p q g"))
                    mv2 = P.tile([64, 2], mt.float32)
                    nc.vector.reduce_sum(mv2[:], gath[:], axis=mybir.AxisListType.X)

                    # k = cst1 / sqrt(var*cst0 + eps); c = cst2 - mu*k
                    m2t = P.tile([64, 1], mt.float32)
                    nc.vector.tensor_tensor(m2t[:], mv2[:, 0:1], mv2[:, 0:1], AO.mult)
                    vart = P.tile([64, 1], mt.float32)
                    nc.vector.tensor_tensor(vart[:], mv2[:, 1:2], m2t[:], AO.subtract)
                    t1 = P.tile([64, 1], mt.float32)
                    nc.vector.tensor_tensor(t1[:], vart[:], cst[:, 0:1], AO.mult)
                    nc.vector.tensor_scalar(t1[:], t1[:], BN_EPS, None, AO.add)
                    sq = P.tile([64, 1], mt.float32)
                    nc.scalar.activation(sq[:], t1[:], AF.Sqrt)
                    rc = P.tile([64, 1], mt.float32)
                    nc.vector.reciprocal(rc[:], sq[:])
                    nc.vector.tensor_tensor(kc[0:64, 0:1], rc[:], cst[:, 1:2], AO.mult)
                    mk = P.tile([64, 1], mt.float32)
                    nc.vector.tensor_tensor(mk[:], mv2[:, 0:1], kc[0:64, 0:1], AO.mult)
                    nc.vector.tensor_tensor(kc[0:64, 1:2], cst[:, 2:3], mk[:],
                                            AO.subtract)
                    nc.sync.dma_start(kc[64:128, :], kc[0:64, :])

                # ---- pass 2: normalize + bypass + store ----
                for s in range(SEGS):
                    bpb = (bpb_tiles.pop(s) if s in bpb_tiles
                           else load_bpb(s))
                    ob = P.tile([128, 7, 448], mt.float32, tag="t2ob", bufs=2,
                                name=f"ob_{s}")
                    nc.scalar.activation(ob[:], y[:, 7 * s:7 * s + 7, :], AF.Identity,
                                         bias=kc[:, 1:2], scale=kc[:, 0:1])
                    nc.vector.tensor_tensor(ob[:], ob[:], bpb[:], AO.add)
                    # store issued from the Pool SWDGE queue: nothing is queued
                    # behind it there, so its wait on the add can't block other work
                    nc.gpsimd.dma_start(out_d.ap()[:, :, 7 * s:7 * s + 7, :], ob[:])


    except _StopBuild:
        pass
    nc.compile()
    return nc


def _get_nc(general_affine):
    key = ("nc", general_affine, NCORES)
    if key not in _cache:
        _cache[key] = _build(general_affine)
    return _cache[key]


def _host_prep(alpha, epsilon, tau, A, weight, gamma, beta):
    import ml_dtypes
    f8 = ml_dtypes.float8_e4m3

    eps_v = np.asarray(epsilon, np.float32).reshape(-1)
    tau_v = np.asarray(tau, np.float32).reshape(-1)
    A_v = np.asarray(A, np.float32).reshape(-1)
    if eps_v.size == 1:
        eps_v = np.full(CIN, eps_v[0], np.float32)
    if tau_v.size == 1:
        tau_v = np.full(CIN, tau_v[0], np.float32)
    if A_v.size == 1:
        A_v = np.full(CIN, A_v[0], np.float32)

    general = not (np.all(eps_v == 0.0) and np.all(tau_v == 1.0))

    w = np.asarray(weight, np.float32)
    scale = np.mean(np.abs(w), axis=(1, 2, 3), dtype=np.float32)
    sw = np.sign(w).astype(np.float32)
    waff = sw * A_v[None, :, None, None]      # fold A (exact for A=+-1 etc.)
    wperm = waff[:, SLOT_TO_CH, :, :]         # [co, slot, kh, kw]
    wp = np.ascontiguousarray(
        np.concatenate([wperm[:, :, 0, :], wperm[:, :, 1, :]], axis=1)
        .transpose(2, 1, 0)).astype(f8)       # [3, 96, 64]
    wsx = np.ascontiguousarray(wperm[:, :, 2, :].transpose(2, 1, 0)).astype(f8)

    sprime = 2.0 * scale
    cst = np.zeros((64, 4), np.float32)
    cst[:, 0] = sprime * sprime
    cst[:, 1] = np.asarray(gamma, np.float32).reshape(-1) * sprime
    cst[:, 2] = np.asarray(beta, np.float32).reshape(-1)

    coef = np.zeros((128, 8), np.float32)
    if general:
        for p in range(128):
            g = p % 16
            for c in range(3):
                ch = 45 + c if g == 15 else 15 * c + g
                coef[p, c] = 1.0 / tau_v[ch]
                coef[p, 3 + c] = -eps_v[ch] / tau_v[ch]
    return general, wp, wsx, cst, coef


def _make_xdev(xi):
    """xi [48, 224, 224] f32 -> [128, 3, 6272] seg-major layout."""
    xr = xi.reshape(CIN, SEGS, SEGQ)
    p = np.arange(128)
    s_idx = p // 16
    g_idx = p % 16
    ch = np.empty((128, 3), np.int64)
    for c in range(3):
        ch[:, c] = np.where(g_idx == 15, 45 + c, 15 * c + g_idx)
    return np.ascontiguousarray(xr[ch, s_idx[:, None], :])


def kernel(x, alpha, epsilon, tau, A, weight, gamma, beta):
    from concourse import bass_utils

    x = np.asarray(x, np.float32)
    general, wp, wsx, cst, coef = _host_prep(alpha, epsilon, tau, A,
                                             weight, gamma, beta)
    nc = _get_nc(general)

    in_maps = []
    for i in range(NCORES):
        xi = np.ascontiguousarray(x[i])
        in_maps.append({
            "xdev": _make_xdev(xi),
            "xch": xi.reshape(CIN, H * W),
            "wp": wp, "ws": wsx, "cst": cst, "coef": coef,
        })
    res = bass_utils.run_bass_kernel_spmd(nc, in_maps,
                                          core_ids=list(range(NCORES)))
    out = np.stack([
        res.results[i]["out"].reshape(2, COUT, NBANK, 2, 224)
        .transpose(1, 2, 0, 3, 4).reshape(COUT, H, W)
        for i in range(NCORES)
    ])
    return out.astype(np.float32)



# revision 13
# speedup vs baseline: 1.5322x; 1.5322x over previous
"""Trainium2 Bass kernel for nn_BiDenseConv2d (binarized 3x3 conv + sync-BN + channel bypass).

Shapes (hardcoded): x [8, 48, 224, 224] f32 -> out [8, 64, 224, 224] f32.
Sharding: data-parallel over batch, 1 image per core; BN stats all-reduced
([64,2] f32 AllGather); weights replicated.

Per-core pipeline (phases overlap via Tile dataflow):
  1. binarize (chunks of [128p=(seg,grp), 14 rows x 226]): rint via fp32 magic
     (Pool), is_ge (DVE), affine {1,0}->+-1 fp8 (ACT); column pads zeroed.
  2. scatter to conv layout xa2f [96, 226, 226] fp8 (A=rows, B=rows+1 copy),
     one 3164B-contiguous DMA per (c, hf, seg); B-half via 16 in-tile copies.
  3. conv: fp8 DoubleRow matmuls, M=128 (low 64 partitions = out rows y0+r,
     high = y0+2+r), N=224, 10 DR per 4-row bank; taps packed via tiles
     (delta in {0,2,4}) x kw over the 2-row stack; +-1 acts x +-1 weights give
     exact integer sums in PSUM f32, evicted to fp16.
  4. BN: sums (DVE evict accum) + sumsq (ACT Square accum); AllGather [64,2];
     k = gamma*s*rsqrt(s^2 var + eps), c = beta - mu*k  (s = mean|w|).
  5. bypass: host supplies xhalf fp16 [64, H*W] (48 identity channels + 16
     3-channel means); loaded per seg straight into the y layout; pass 2:
     ob = affine(y) (ACT) + byp (DVE), stored via Pool SWDGE.

Conv input channel at slot 16c+g is channel 15c+g (g<15) / 45+c (g=15),
folded into the weights host-side. Output layout matches the baseline.
"""
import sys
import numpy as np

sys.path.insert(0, '/opt/trn_rl_repo')

B, CIN, COUT, H, W = 8, 48, 64, 224, 224
NCORES = 8
SEGS, SEGR = 8, 28
PW = 226
RQ = 14 * PW            # 3164 elems per (c, hf) chunk row-block
NBANK = 56
BN_EPS = 1e-5
MAGIC = 12582912.0

_cache = {}

SLOT_TO_CH = np.zeros(48, np.int64)
for _c in range(3):
    for _g in range(16):
        SLOT_TO_CH[16 * _c + _g] = (45 + _c) if _g == 15 else (15 * _c + _g)

# DR tile pairs (delta, kw); 'z' = zero-weight tile (arbitrary in-bounds read)
DR_PAIRS = [((0, 0), (2, 0)),
            ((0, 1), (2, 1)),
            ((0, 2), (4, 0)),
            ((2, 2), (4, 1)),
            ((1, 0, 'z'), (4, 2))]


def _build(general_affine: bool):
    from concourse import bacc, tile, mybir
    from concourse.ap import AP
    mt = mybir.dt
    AO = mybir.AluOpType
    AF = mybir.ActivationFunctionType

    nc = bacc.Bacc("TRN2", target_bir_lowering=False, debug=False,
                   num_devices=NCORES)

    xdev_d = nc.dram_tensor("xdev", [128, 2, 3, RQ], mt.float32,
                            kind="ExternalInput")
    xhalf_d = nc.dram_tensor("xhalf", [COUT, H * W], mt.float16,
                             kind="ExternalInput")
    wdr_d = nc.dram_tensor("wdr", [96, 5 * 2 * 128], mt.float8e4,
                           kind="ExternalInput")
    cst_d = nc.dram_tensor("cst", [64, 4], mt.float32, kind="ExternalInput")
    coef_d = nc.dram_tensor("coef", [128, 8], mt.float32, kind="ExternalInput")
    out_d = nc.dram_tensor("out", [2, COUT, NBANK, 448], mt.float32,
                           kind="ExternalOutput")

    with tile.TileContext(nc) as tc:
        with tc.tile_pool(name="main", bufs=1) as P, \
             tc.tile_pool(name="psum", bufs=6, space="PSUM") as PS, \
             tc.tile_pool(name="dram", bufs=1, space="DRAM") as D:

            # ---- constants ----
            wdr = P.tile([96, 5, 2, 128], mt.float8e4)
            nc.sync.dma_start(
                wdr[:], wdr_d.ap().rearrange("p (d t m) -> p d t m", d=5, t=2))
            cst = P.tile([64, 4], mt.float32)
            nc.sync.dma_start(cst[:], cst_d.ap())
            coef = P.tile([128, 8], mt.float32)
            if general_affine:
                nc.sync.dma_start(coef[:], coef_d.ap())

            # ---- persistent tiles ----
            xa2f = P.tile([96, PW, PW], mt.float8e4)
            y = P.tile([128, NBANK, 448], mt.float16)
            sums = P.tile([128, NBANK], mt.float32)
            sqs = P.tile([128, NBANK], mt.float32)

            neg1 = P.tile([128, 1], mt.float32)
            nc.vector.memset(neg1[:], -1.0)
            two = P.tile([128, 1], mt.float32)
            nc.vector.memset(two[:], 2.0)

            # top/bottom pads; A row 224 is rewritten by the (h1, s7) scatter
            nc.vector.memset(xa2f[0:96, 0, :], 0.0)
            nc.vector.memset(xa2f[0:96, 224:226, :], 0.0)

            # ---- binarize + scatter, chunk order (hf, c) ----
            for hf in range(2):
                for c in range(3):
                    x1b = P.tile([128, RQ], mt.float32, tag="x1", bufs=2,
                                 name=f"x1b_{hf}_{c}")
                    nc.sync.dma_start(x1b[:], xdev_d.ap()[:, hf, c, :])
                    if general_affine:
                        nc.vector.tensor_scalar(
                            x1b[:], x1b[:], coef[:, c:c + 1],
                            coef[:, 3 + c:4 + c], AO.mult, AO.add)
                    m1 = P.tile([128, RQ], mt.bfloat16, tag="m1", bufs=2,
                                name=f"m1_{hf}_{c}")
                    nc.gpsimd.tensor_scalar(m1[:], x1b[:], MAGIC, MAGIC,
                                            AO.add, AO.subtract)
                    t2b = P.tile([128, RQ], mt.float8e4, tag="t2b", bufs=2,
                                 name=f"t2b_{hf}_{c}")
                    nc.vector.tensor_tensor(t2b[:], x1b[:], m1[:], AO.is_ge)
                    xa1b = P.tile([128, 14, PW], mt.float8e4, tag="xa1b", bufs=2,
                                  name=f"xa1b_{hf}_{c}")
                    nc.scalar.activation(
                        xa1b[:].rearrange("p a b -> p (a b)"), t2b[:],
                        AF.Identity, bias=neg1[:], scale=two[:])
                    # zero the column pads
                    nc.vector.memset(xa1b[:, :, 0], 0.0)
                    nc.vector.memset(xa1b[:, :, 225], 0.0)
                    # scatter: per seg, 16 partitions, 3164B contiguous
                    for s in range(SEGS):
                        r0 = 28 * s + 14 * hf + 1
                        nc.scalar.dma_start(
                            xa2f[16 * c:16 * c + 16, r0:r0 + 14, :],
                            xa1b[16 * s:16 * s + 16, :, :])
                # B-half copies for this hf: B[r] = A[r+1]
                for s in range(SEGS):
                    r0 = 28 * s + 14 * hf
                    nc.scalar.dma_start(
                        xa2f[48:96, r0:r0 + 14, :],
                        xa2f[0:48, r0 + 1:r0 + 15, :])

            # ---- conv: DoubleRow matmuls ----
            xbase = xa2f[0:96, 0:1, 0:1]
            pstride = int(xbase.ap[0][0])
            xoff = int(xbase.offset)

            # bank emission order: phase A (ready after hf0), then per-seg rest
            order = [(0, 0)] + [(s, j) for s in range(SEGS) for j in (1, 2)]
            for s in range(SEGS):
                order += [(s, 3), (s, 4), (s, 5), (s, 6)]
                if s + 1 < SEGS:
                    order.append((s + 1, 0))

            perf = mybir.MatmulPerfMode.DoubleRow
            for (s, j) in order:
                b = 7 * s + j
                y0 = 4 * b
                ps = PS.tile([128, 448], mt.float32, tag="ps", name=f"ps_{b}")
                for r in range(2):
                    for d, (t0, t1) in enumerate(DR_PAIRS):
                        o0 = (y0 + r + t0[0]) * PW + t0[1]
                        o1 = (y0 + r + t1[0]) * PW + t1[1]
                        mv = AP(xbase.tensor, xoff + o0,
                                [[pstride, 96], [o1 - o0, 2], [1, 224]])
                        nc.tensor.matmul(ps[:, 224 * r:224 * r + 224],
                                         wdr[:, d, :, :], mv,
                                         start=(d == 0), stop=(d == 4),
                                         perf_mode=perf)
                nc.vector.tensor_scalar(y[:, b, :], ps[:], 1.0, None,
                                        AO.mult, AO.add,
                                        accum_out=sums[:, b:b + 1])
                nc.scalar.activation(ps[:], ps[:], AF.Square,
                                     accum_out=sqs[:, b:b + 1])

            # ---- bypass loads (fp16, straight into y layout) ----
            byp_tiles = {}

            def load_byp(s):
                bp = P.tile([128, 7, 448], mt.float16, tag="byp", bufs=4,
                            name=f"byp_{s}")
                for ci in range(2):
                    src = AP(xhalf_d.ap().tensor, 6272 * s + 448 * ci,
                             [[H * W, COUT], [896, 7], [1, 448]])
                    nc.sync.dma_start(bp[64 * ci:64 * ci + 64, :, :], src)
                return bp

            for s in range(4):
                byp_tiles[s] = load_byp(s)

            # ---- stats + collective + BN affine ----
            kc = P.tile([128, 2], mt.float32)
            ssum = P.tile([128, 1], mt.float32)
            ssq = P.tile([128, 1], mt.float32)
            nc.vector.reduce_sum(ssum[:], sums[:], axis=mybir.AxisListType.X)
            nc.vector.reduce_sum(ssq[:], sqs[:], axis=mybir.AxisListType.X)
            toph = P.tile([64, 2], mt.float32)
            nc.scalar.dma_start(toph[:, 0:1], ssum[64:128, :])
            nc.scalar.dma_start(toph[:, 1:2], ssq[64:128, :])
            cb = P.tile([64, 2], mt.float32)
            nc.vector.tensor_tensor(cb[:, 0:1], ssum[0:64, :], toph[:, 0:1],
                                    AO.add)
            nc.vector.tensor_tensor(cb[:, 1:2], ssq[0:64, :], toph[:, 1:2],
                                    AO.add)
            nc.vector.tensor_scalar(cb[:], cb[:], 1.0 / float(B * H * W),
                                    None, AO.mult)
            cbin = D.tile([64, 2], mt.float32)
            cbout = D.tile([NCORES, 64, 2], mt.float32)
            nc.scalar.dma_start(cbin[:], cb[:])
            nc.gpsimd.collective_compute(
                "AllGather", mybir.AluOpType.bypass,
                replica_groups=[list(range(NCORES))],
                ins=[cbin.opt()], outs=[cbout.opt()])
            gath = P.tile([64, 2, NCORES], mt.float32)
            nc.scalar.dma_start(gath[:], cbout[:].rearrange("g p q -> p q g"))
            mv2 = P.tile([64, 2], mt.float32)
            nc.vector.reduce_sum(mv2[:], gath[:], axis=mybir.AxisListType.X)

            m2t = P.tile([64, 1], mt.float32)
            nc.vector.tensor_tensor(m2t[:], mv2[:, 0:1], mv2[:, 0:1], AO.mult)
            vart = P.tile([64, 1], mt.float32)
            nc.vector.tensor_tensor(vart[:], mv2[:, 1:2], m2t[:], AO.subtract)
            t1 = P.tile([64, 1], mt.float32)
            nc.vector.tensor_tensor(t1[:], vart[:], cst[:, 0:1], AO.mult)
            nc.vector.tensor_scalar(t1[:], t1[:], BN_EPS, None, AO.add)
            sq = P.tile([64, 1], mt.float32)
            nc.scalar.activation(sq[:], t1[:], AF.Sqrt)
            rc = P.tile([64, 1], mt.float32)
            nc.vector.reciprocal(rc[:], sq[:])
            nc.vector.tensor_tensor(kc[0:64, 0:1], rc[:], cst[:, 1:2], AO.mult)
            mk = P.tile([64, 1], mt.float32)
            nc.vector.tensor_tensor(mk[:], mv2[:, 0:1], kc[0:64, 0:1], AO.mult)
            nc.vector.tensor_tensor(kc[0:64, 1:2], cst[:, 2:3], mk[:],
                                    AO.subtract)
            nc.scalar.dma_start(kc[64:128, :], kc[0:64, :])

            # ---- pass 2: affine + bypass + store ----
            for s in range(SEGS):
                bp = byp_tiles.pop(s) if s in byp_tiles else load_byp(s)
                ob = P.tile([128, 7, 448], mt.float32, tag="ob", bufs=2,
                            name=f"ob_{s}")
                nc.scalar.activation(ob[:], y[:, 7 * s:7 * s + 7, :],
                                     AF.Identity, bias=kc[:, 1:2],
                                     scale=kc[:, 0:1])
                nc.vector.tensor_tensor(ob[:], ob[:], bp[:], AO.add)
                nc.gpsimd.dma_start(out_d.ap()[:, :, 7 * s:7 * s + 7, :], ob[:])

    nc.compile()
    return nc


def _get_nc(general_affine):
    key = ("nc", general_affine, NCORES)
    if key not in _cache:
        _cache[key] = _build(general_affine)
    return _cache[key]


def _pack_weights(wt):
    """wt [64, 48, 3, 3] (+-1 * A, slot-permuted) -> [96, 5, 2, 128] f32."""
    w = np.zeros((96, 5, 2, 128), np.float32)
    covered = set()
    for d, pair in enumerate(DR_PAIRS):
        for t, tl in enumerate(pair):
            if len(tl) == 3:
                continue
            delta, kw = tl
            for stack in (0, 1):
                for half, rho in ((0, 0), (1, 2)):
                    kh = delta + stack - rho
                    if 0 <= kh <= 2 and (rho, kh, kw) not in covered:
                        covered.add((rho, kh, kw))
                        w[48 * stack:48 * stack + 48, d, t,
                          64 * half:64 * half + 64] = wt[:, :, kh, kw].T
    assert len(covered) == 18
    return w


def _host_prep(alpha, epsilon, tau, A, weight, gamma, beta):
    import ml_dtypes
    f8 = ml_dtypes.float8_e4m3

    eps_v = np.asarray(epsilon, np.float32).reshape(-1)
    tau_v = np.asarray(tau, np.float32).reshape(-1)
    A_v = np.asarray(A, np.float32).reshape(-1)
    if eps_v.size == 1:
        eps_v = np.full(CIN, eps_v[0], np.float32)
    if tau_v.size == 1:
        tau_v = np.full(CIN, tau_v[0], np.float32)
    if A_v.size == 1:
        A_v = np.full(CIN, A_v[0], np.float32)

    general = not (np.all(eps_v == 0.0) and np.all(tau_v == 1.0))

    w = np.asarray(weight, np.float32)
    scale = np.mean(np.abs(w), axis=(1, 2, 3), dtype=np.float32)
    waff = np.sign(w) * A_v[None, :, None, None]
    wperm = waff[:, SLOT_TO_CH, :, :]
    wdr = _pack_weights(wperm).reshape(96, -1).astype(f8)

    cst = np.zeros((64, 4), np.float32)
    cst[:, 0] = scale * scale
    cst[:, 1] = np.asarray(gamma, np.float32).reshape(-1) * scale
    cst[:, 2] = np.asarray(beta, np.float32).reshape(-1)

    coef = np.zeros((128, 8), np.float32)
    if general:
        for p in range(128):
            g = p % 16
            for c in range(3):
                ch = 45 + c if g == 15 else 15 * c + g
                coef[p, c] = 1.0 / tau_v[ch]
                coef[p, 3 + c] = -eps_v[ch] / tau_v[ch]
    return general, wdr, cst, coef


def _make_xdev(xi):
    """xi [48, 224, 224] f32 -> [128, 2, 3, 3164] (rows padded to 226)."""
    xp = np.zeros((CIN, H, PW), np.float32)
    xp[:, :, 1:225] = xi
    xr = xp.reshape(CIN, SEGS, 2, RQ)       # [ch, seg, hf, 14*226]
    p = np.arange(128)
    s_idx, g_idx = p // 16, p % 16
    out = np.empty((128, 2, 3, RQ), np.float32)
    for c in range(3):
        ch = np.where(g_idx == 15, 45 + c, 15 * c + g_idx)
        out[:, :, c, :] = xr[ch, s_idx, :, :]
    return out


def _make_xhalf(xi):
    """xi [48, 224, 224] f32 -> [64, H*W] fp16 (identity + 16 group means)."""
    xh = np.empty((COUT, H * W), np.float16)
    xh[0:CIN] = xi.reshape(CIN, -1).astype(np.float16)
    xf = xi.reshape(CIN, -1)
    xh[48:63] = xf[0:45].reshape(3, 15, -1).mean(axis=0,
                                                 dtype=np.float32).astype(np.float16)
    xh[63] = xf[45:48].mean(axis=0, dtype=np.float32).astype(np.float16)
    return xh


def kernel(x, alpha, epsilon, tau, A, weight, gamma, beta):
    from concourse import bass_utils

    x = np.asarray(x, np.float32)
    general, wdr, cst, coef = _host_prep(alpha, epsilon, tau, A,
                                         weight, gamma, beta)
    nc = _get_nc(general)

    in_maps = []
    for i in range(NCORES):
        xi = np.ascontiguousarray(x[i])
        in_maps.append({
            "xdev": _make_xdev(xi),
            "xhalf": _make_xhalf(xi),
            "wdr": wdr, "cst": cst, "coef": coef,
        })
    res = bass_utils.run_bass_kernel_spmd(nc, in_maps,
                                          core_ids=list(range(NCORES)))
    out = np.stack([
        res.results[i]["out"].reshape(2, COUT, NBANK, 2, 224)
        .transpose(1, 2, 0, 3, 4).reshape(COUT, H, W)
        for i in range(NCORES)
    ])
    return out.astype(np.float32)


# revision 16
# speedup vs baseline: 1.5824x; 1.0328x over previous
"""Trainium2 Bass kernel for nn_BiDenseConv2d (binarized 3x3 conv + sync-BN + channel bypass).

Shapes (hardcoded): x [8, 48, 224, 224] f32 -> out [8, 64, 224, 224] f32.
Sharding: data-parallel over batch, 1 image per core; BN stats all-reduced
([64,2] f32 AllGather); weights replicated.

Per-core pipeline (phases overlap via Tile dataflow):
  1. binarize (chunks of [128p=(seg,grp), 14 rows x 226]): rint via fp32 magic
     (Pool), is_ge (DVE), affine {1,0}->+-1 fp8 (ACT); column pads zeroed.
  2. scatter to conv layout xa2f [96, 226, 226] fp8 (A=rows, B=rows+1 copy),
     one 3164B-contiguous DMA per (c, hf, seg); B-half via 16 in-tile copies.
  3. conv: fp8 DoubleRow matmuls, M=128 (low 64 partitions = out rows y0+r,
     high = y0+2+r), N=224, 10 DR per 4-row bank; taps packed via tiles
     (delta in {0,2,4}) x kw over the 2-row stack; +-1 acts x +-1 weights give
     exact integer sums in PSUM f32, evicted to fp16.
  4. BN: sums (DVE evict accum) + sumsq (ACT Square accum); AllGather [64,2];
     k = gamma*s*rsqrt(s^2 var + eps), c = beta - mu*k  (s = mean|w|).
  5. bypass: host supplies xhalf fp16 [64, H*W] (48 identity channels + 16
     3-channel means); loaded per seg straight into the y layout; pass 2:
     ob = affine(y) (ACT) + byp (DVE), stored via Pool SWDGE.

Conv input channel at slot 16c+g is channel 15c+g (g<15) / 45+c (g=15),
folded into the weights host-side. Output layout matches the baseline.
"""
import sys
import numpy as np

sys.path.insert(0, '/opt/trn_rl_repo')

B, CIN, COUT, H, W = 8, 48, 64, 224, 224
NCORES = 8
SEGS, SEGR = 8, 28
PW = 226
RQ = 14 * PW            # 3164 elems per (c, hf) chunk row-block
NBANK = 56
BN_EPS = 1e-5
MAGIC = 12582912.0

_cache = {}

SLOT_TO_CH = np.zeros(48, np.int64)
for _c in range(3):
    for _g in range(16):
        SLOT_TO_CH[16 * _c + _g] = (45 + _c) if _g == 15 else (15 * _c + _g)

# DR tile pairs (delta, kw); 'z' = zero-weight tile (arbitrary in-bounds read)
DR_PAIRS = [((0, 0), (2, 0)),
            ((0, 1), (2, 1)),
            ((0, 2), (4, 0)),
            ((2, 2), (4, 1)),
            ((1, 0, 'z'), (4, 2))]


def _build(general_affine: bool):
    from concourse import bacc, tile, mybir
    from concourse.ap import AP
    mt = mybir.dt
    AO = mybir.AluOpType
    AF = mybir.ActivationFunctionType

    nc = bacc.Bacc("TRN2", target_bir_lowering=False, debug=False,
                   num_devices=NCORES)

    xdev_d = nc.dram_tensor("xdev", [128, 2, 3, RQ], mt.float32,
                            kind="ExternalInput")
    xhalf_d = nc.dram_tensor("xhalf", [COUT, H * W], mt.float16,
                             kind="ExternalInput")
    wdr_d = nc.dram_tensor("wdr", [96, 5 * 2 * 128], mt.float8e4,
                           kind="ExternalInput")
    cst_d = nc.dram_tensor("cst", [64, 4], mt.float32, kind="ExternalInput")
    coef_d = nc.dram_tensor("coef", [128, 8], mt.float32, kind="ExternalInput")
    out_d = nc.dram_tensor("out", [2, COUT, NBANK, 448], mt.float32,
                           kind="ExternalOutput")

    with tile.TileContext(nc) as tc:
        with tc.tile_pool(name="main", bufs=1) as P, \
             tc.tile_pool(name="psum", bufs=6, space="PSUM") as PS, \
             tc.tile_pool(name="dram", bufs=1, space="DRAM") as D:

            # ---- constants ----
            wdr = P.tile([96, 5, 2, 128], mt.float8e4)
            nc.sync.dma_start(
                wdr[:], wdr_d.ap().rearrange("p (d t m) -> p d t m", d=5, t=2))
            cst = P.tile([64, 4], mt.float32)
            nc.sync.dma_start(cst[:], cst_d.ap())
            coef = P.tile([128, 8], mt.float32)
            if general_affine:
                nc.sync.dma_start(coef[:], coef_d.ap())

            # ---- persistent tiles ----
            xa2f = P.tile([96, PW, PW], mt.float8e4)
            y = P.tile([128, NBANK, 448], mt.float16)
            sums = P.tile([128, NBANK], mt.float32)
            sqs = P.tile([128, NBANK], mt.float32)

            neg1 = P.tile([128, 1], mt.float32)
            nc.vector.memset(neg1[:], -1.0)
            two = P.tile([128, 1], mt.float32)
            nc.vector.memset(two[:], 2.0)

            # top/bottom pads; A row 224 is rewritten by the (h1, s7) scatter
            nc.vector.memset(xa2f[0:96, 0, :], 0.0)
            nc.vector.memset(xa2f[0:96, 224:226, :], 0.0)

            # ---- binarize + scatter, chunk order (hf, c) ----
            # all loads issued first so the SP queue never head-blocks them
            x1bs = {}
            for hf in range(2):
                for c in range(3):
                    x1b = P.tile([128, RQ], mt.float32, tag="x1", bufs=2,
                                 name=f"x1b_{hf}_{c}")
                    nc.sync.dma_start(x1b[:], xdev_d.ap()[:, hf, c, :])
                    x1bs[(hf, c)] = x1b
            for hf in range(2):
                for c in range(3):
                    x1b = x1bs[(hf, c)]
                    if general_affine:
                        nc.vector.tensor_scalar(
                            x1b[:], x1b[:], coef[:, c:c + 1],
                            coef[:, 3 + c:4 + c], AO.mult, AO.add)
                    m1 = P.tile([128, RQ], mt.bfloat16, tag="m1", bufs=2,
                                name=f"m1_{hf}_{c}")
                    rint_eng = nc.gpsimd if (3 * hf + c) % 2 == 0 else nc.vector
                    rint_eng.tensor_scalar(m1[:], x1b[:], MAGIC, MAGIC,
                                           AO.add, AO.subtract)
                    t2b = P.tile([128, RQ], mt.float8e4, tag="t2b", bufs=2,
                                 name=f"t2b_{hf}_{c}")
                    nc.vector.tensor_tensor(t2b[:], x1b[:], m1[:], AO.is_ge)
                    xa1b = P.tile([128, 14, PW], mt.float8e4, tag="xa1b", bufs=2,
                                  name=f"xa1b_{hf}_{c}")
                    nc.scalar.activation(
                        xa1b[:].rearrange("p a b -> p (a b)"), t2b[:],
                        AF.Identity, bias=neg1[:], scale=two[:])
                    # zero the column pads
                    nc.gpsimd.memset(xa1b[:, :, 0], 0.0)
                    nc.gpsimd.memset(xa1b[:, :, 225], 0.0)
                    # scatter: per seg, 16 partitions, 3164B contiguous
                    for s in range(SEGS):
                        r0 = 28 * s + 14 * hf + 1
                        nc.sync.dma_start(
                            xa2f[16 * c:16 * c + 16, r0:r0 + 14, :],
                            xa1b[16 * s:16 * s + 16, :, :])
                # B-half copies for this hf: B[r] = A[r+1]
                for s in range(SEGS):
                    r0 = 28 * s + 14 * hf
                    nc.sync.dma_start(
                        xa2f[48:96, r0:r0 + 14, :],
                        xa2f[0:48, r0 + 1:r0 + 15, :])

            # ---- conv: DoubleRow matmuls ----
            xbase = xa2f[0:96, 0:1, 0:1]
            pstride = int(xbase.ap[0][0])
            xoff = int(xbase.offset)

            # bank emission order: phase A (ready after hf0), then per-seg rest
            order = [(0, 0)] + [(s, j) for s in range(SEGS) for j in (1, 2)]
            for s in range(SEGS):
                order += [(s, 3), (s, 4), (s, 5), (s, 6)]
                if s + 1 < SEGS:
                    order.append((s + 1, 0))

            perf = mybir.MatmulPerfMode.DoubleRow
            for (s, j) in order:
                b = 7 * s + j
                y0 = 4 * b
                ps = PS.tile([128, 448], mt.float32, tag="ps", name=f"ps_{b}")
                for r in range(2):
                    for d, (t0, t1) in enumerate(DR_PAIRS):
                        o0 = (y0 + r + t0[0]) * PW + t0[1]
                        o1 = (y0 + r + t1[0]) * PW + t1[1]
                        mv = AP(xbase.tensor, xoff + o0,
                                [[pstride, 96], [o1 - o0, 2], [1, 224]])
                        nc.tensor.matmul(ps[:, 224 * r:224 * r + 224],
                                         wdr[:, d, :, :], mv,
                                         start=(d == 0), stop=(d == 4),
                                         perf_mode=perf)
                nc.vector.tensor_scalar(y[:, b, :], ps[:], 1.0, None,
                                        AO.mult, AO.add,
                                        accum_out=sums[:, b:b + 1])
                nc.scalar.activation(ps[:], ps[:], AF.Square,
                                     accum_out=sqs[:, b:b + 1])

            # ---- bypass loads (fp16, straight into y layout) ----
            byp_tiles = {}

            def load_byp(s):
                bp = P.tile([128, 7, 448], mt.float16, tag="byp", bufs=4,
                            name=f"byp_{s}")
                for ci in range(2):
                    src = AP(xhalf_d.ap().tensor, 6272 * s + 448 * ci,
                             [[H * W, COUT], [896, 7], [1, 448]])
                    nc.sync.dma_start(bp[64 * ci:64 * ci + 64, :, :], src)
                return bp

            for s in range(4):
                byp_tiles[s] = load_byp(s)

            # ---- stats + collective + BN affine ----
            kc = P.tile([128, 2], mt.float32)
            ssum = P.tile([128, 1], mt.float32)
            ssq = P.tile([128, 1], mt.float32)
            nc.vector.reduce_sum(ssum[:], sums[:], axis=mybir.AxisListType.X)
            nc.vector.reduce_sum(ssq[:], sqs[:], axis=mybir.AxisListType.X)
            toph = P.tile([64, 2], mt.float32)
            nc.scalar.dma_start(toph[:, 0:1], ssum[64:128, :])
            nc.scalar.dma_start(toph[:, 1:2], ssq[64:128, :])
            cb = P.tile([64, 2], mt.float32)
            nc.vector.tensor_tensor(cb[:, 0:1], ssum[0:64, :], toph[:, 0:1],
                                    AO.add)
            nc.vector.tensor_tensor(cb[:, 1:2], ssq[0:64, :], toph[:, 1:2],
                                    AO.add)
            nc.vector.tensor_scalar(cb[:], cb[:], 1.0 / float(B * H * W),
                                    None, AO.mult)
            cbin = D.tile([64, 2], mt.float32)
            cbout = D.tile([NCORES, 64, 2], mt.float32)
            nc.scalar.dma_start(cbin[:], cb[:])
            nc.gpsimd.collective_compute(
                "AllGather", mybir.AluOpType.bypass,
                replica_groups=[list(range(NCORES))],
                ins=[cbin.opt()], outs=[cbout.opt()])
            gath = P.tile([64, 2, NCORES], mt.float32)
            nc.scalar.dma_start(gath[:], cbout[:].rearrange("g p q -> p q g"))
            mv2 = P.tile([64, 2], mt.float32)
            nc.vector.reduce_sum(mv2[:], gath[:], axis=mybir.AxisListType.X)

            m2t = P.tile([64, 1], mt.float32)
            nc.vector.tensor_tensor(m2t[:], mv2[:, 0:1], mv2[:, 0:1], AO.mult)
            vart = P.tile([64, 1], mt.float32)
            nc.vector.tensor_tensor(vart[:], mv2[:, 1:2], m2t[:], AO.subtract)
            t1 = P.tile([64, 1], mt.float32)
            nc.vector.tensor_tensor(t1[:], vart[:], cst[:, 0:1], AO.mult)
            nc.vector.tensor_scalar(t1[:], t1[:], BN_EPS, None, AO.add)
            sq = P.tile([64, 1], mt.float32)
            nc.scalar.activation(sq[:], t1[:], AF.Sqrt)
            rc = P.tile([64, 1], mt.float32)
            nc.vector.reciprocal(rc[:], sq[:])
            nc.vector.tensor_tensor(kc[0:64, 0:1], rc[:], cst[:, 1:2], AO.mult)
            mk = P.tile([64, 1], mt.float32)
            nc.vector.tensor_tensor(mk[:], mv2[:, 0:1], kc[0:64, 0:1], AO.mult)
            nc.vector.tensor_tensor(kc[0:64, 1:2], cst[:, 2:3], mk[:],
                                    AO.subtract)
            nc.scalar.dma_start(kc[64:128, :], kc[0:64, :])

            # ---- pass 2: affine + bypass + store ----
            for s in range(SEGS):
                bp = byp_tiles.pop(s) if s in byp_tiles else load_byp(s)
                ob = P.tile([128, 7, 448], mt.float32, tag="ob", bufs=2,
                            name=f"ob_{s}")
                nc.scalar.activation(ob[:], y[:, 7 * s:7 * s + 7, :],
                                     AF.Identity, bias=kc[:, 1:2],
                                     scale=kc[:, 0:1])
                nc.vector.tensor_tensor(ob[:], ob[:], bp[:], AO.add)
                nc.gpsimd.dma_start(out_d.ap()[:, :, 7 * s:7 * s + 7, :], ob[:])

    nc.compile()
    return nc


def _get_nc(general_affine):
    key = ("nc", general_affine, NCORES)
    if key not in _cache:
        _cache[key] = _build(general_affine)
    return _cache[key]


def _pack_weights(wt):
    """wt [64, 48, 3, 3] (+-1 * A, slot-permuted) -> [96, 5, 2, 128] f32."""
    w = np.zeros((96, 5, 2, 128), np.float32)
    covered = set()
    for d, pair in enumerate(DR_PAIRS):
        for t, tl in enumerate(pair):
            if len(tl) == 3:
                continue
            delta, kw = tl
            for stack in (0, 1):
                for half, rho in ((0, 0), (1, 2)):
                    kh = delta + stack - rho
                    if 0 <= kh <= 2 and (rho, kh, kw) not in covered:
                        covered.add((rho, kh, kw))
                        w[48 * stack:48 * stack + 48, d, t,
                          64 * half:64 * half + 64] = wt[:, :, kh, kw].T
    assert len(covered) == 18
    return w


def _host_prep(alpha, epsilon, tau, A, weight, gamma, beta):
    import ml_dtypes
    f8 = ml_dtypes.float8_e4m3

    eps_v = np.asarray(epsilon, np.float32).reshape(-1)
    tau_v = np.asarray(tau, np.float32).reshape(-1)
    A_v = np.asarray(A, np.float32).reshape(-1)
    if eps_v.size == 1:
        eps_v = np.full(CIN, eps_v[0], np.float32)
    if tau_v.size == 1:
        tau_v = np.full(CIN, tau_v[0], np.float32)
    if A_v.size == 1:
        A_v = np.full(CIN, A_v[0], np.float32)

    general = not (np.all(eps_v == 0.0) and np.all(tau_v == 1.0))

    w = np.asarray(weight, np.float32)
    scale = np.mean(np.abs(w), axis=(1, 2, 3), dtype=np.float32)
    waff = np.sign(w) * A_v[None, :, None, None]
    wperm = waff[:, SLOT_TO_CH, :, :]
    wdr = _pack_weights(wperm).reshape(96, -1).astype(f8)

    cst = np.zeros((64, 4), np.float32)
    cst[:, 0] = scale * scale
    cst[:, 1] = np.asarray(gamma, np.float32).reshape(-1) * scale
    cst[:, 2] = np.asarray(beta, np.float32).reshape(-1)

    coef = np.zeros((128, 8), np.float32)
    if general:
        for p in range(128):
            g = p % 16
            for c in range(3):
                ch = 45 + c if g == 15 else 15 * c + g
                coef[p, c] = 1.0 / tau_v[ch]
                coef[p, 3 + c] = -eps_v[ch] / tau_v[ch]
    return general, wdr, cst, coef


def _make_xdev(xi):
    """xi [48, 224, 224] f32 -> [128, 2, 3, 3164] (rows padded to 226)."""
    xp = np.zeros((CIN, H, PW), np.float32)
    xp[:, :, 1:225] = xi
    xr = xp.reshape(CIN, SEGS, 2, RQ)       # [ch, seg, hf, 14*226]
    p = np.arange(128)
    s_idx, g_idx = p // 16, p % 16
    out = np.empty((128, 2, 3, RQ), np.float32)
    for c in range(3):
        ch = np.where(g_idx == 15, 45 + c, 15 * c + g_idx)
        out[:, :, c, :] = xr[ch, s_idx, :, :]
    return out


def _make_xhalf(xi):
    """xi [48, 224, 224] f32 -> [64, H*W] fp16 (identity + 16 group means)."""
    xh = np.empty((COUT, H * W), np.float16)
    xh[0:CIN] = xi.reshape(CIN, -1).astype(np.float16)
    xf = xi.reshape(CIN, -1)
    xh[48:63] = xf[0:45].reshape(3, 15, -1).mean(axis=0,
                                                 dtype=np.float32).astype(np.float16)
    xh[63] = xf[45:48].mean(axis=0, dtype=np.float32).astype(np.float16)
    return xh


def kernel(x, alpha, epsilon, tau, A, weight, gamma, beta):
    from concourse import bass_utils

    x = np.asarray(x, np.float32)
    general, wdr, cst, coef = _host_prep(alpha, epsilon, tau, A,
                                         weight, gamma, beta)
    nc = _get_nc(general)

    in_maps = []
    for i in range(NCORES):
        xi = np.ascontiguousarray(x[i])
        in_maps.append({
            "xdev": _make_xdev(xi),
            "xhalf": _make_xhalf(xi),
            "wdr": wdr, "cst": cst, "coef": coef,
        })
    res = bass_utils.run_bass_kernel_spmd(nc, in_maps,
                                          core_ids=list(range(NCORES)))
    out = np.stack([
        res.results[i]["out"].reshape(2, COUT, NBANK, 2, 224)
        .transpose(1, 2, 0, 3, 4).reshape(COUT, H, W)
        for i in range(NCORES)
    ])
    return out.astype(np.float32)


# revision 23
# speedup vs baseline: 1.7990x; 1.1368x over previous
"""Trainium2 Bass kernel for nn_BiDenseConv2d (binarized 3x3 conv + sync-BN + channel bypass).

Shapes (hardcoded): x [8, 48, 224, 224] f32 -> out [8, 64, 224, 224] f32.
Sharding: data-parallel over batch, 1 image per core; BN stats all-reduced
([64,2] f32 AllGather); weights replicated.

Per-core pipeline (phases overlap via Tile dataflow):
  1. binarize (chunks of [128p=(seg,grp), 14 rows x 226]): rint via fp32 magic
     (Pool), is_ge (DVE), affine {1,0}->+-1 fp8 (ACT); column pads zeroed.
  2. scatter to conv layout xa2f [96, 226, 226] fp8 (A=rows, B=rows+1 copy),
     one 3164B-contiguous DMA per (c, hf, seg); B-half via 16 in-tile copies.
  3. conv: fp8 DoubleRow matmuls, M=128 (low 64 partitions = out rows y0+r,
     high = y0+2+r), N=224, 10 DR per 4-row bank; taps packed via tiles
     (delta in {0,2,4}) x kw over the 2-row stack; +-1 acts x +-1 weights give
     exact integer sums in PSUM f32, evicted to fp16.
  4. BN: sums (DVE evict accum) + sumsq (ACT Square accum); AllGather [64,2];
     k = gamma*s*rsqrt(s^2 var + eps), c = beta - mu*k  (s = mean|w|).
  5. bypass: host supplies xhalf fp16 [64, H*W] (48 identity channels + 16
     3-channel means); loaded per seg straight into the y layout; pass 2:
     ob = affine(y) (ACT) + byp (DVE), stored via Pool SWDGE.

Conv input channel at slot 16c+g is channel 15c+g (g<15) / 45+c (g=15),
folded into the weights host-side. Output layout matches the baseline.
"""
import sys
import numpy as np

sys.path.insert(0, '/opt/trn_rl_repo')

B, CIN, COUT, H, W = 8, 48, 64, 224, 224
NCORES = 8
SEGS, SEGR = 8, 28
PW = 226
RQ = 14 * PW            # 3164 elems per (c, hf) chunk row-block
NBANK = 56
BN_EPS = 1e-5
MAGIC = 12582912.0

_cache = {}

SLOT_TO_CH = np.zeros(48, np.int64)
for _c in range(3):
    for _g in range(16):
        SLOT_TO_CH[16 * _c + _g] = (45 + _c) if _g == 15 else (15 * _c + _g)

# DR tile pairs (delta, kw); 'z' = zero-weight tile (arbitrary in-bounds read)
DR_PAIRS = [((0, 0), (2, 0)),
            ((0, 1), (2, 1)),
            ((0, 2), (4, 0)),
            ((2, 2), (4, 1)),
            ((1, 0, 'z'), (4, 2))]


def _build(general_affine: bool):
    from concourse import bacc, tile, mybir
    from concourse.ap import AP
    mt = mybir.dt
    AO = mybir.AluOpType
    AF = mybir.ActivationFunctionType

    nc = bacc.Bacc("TRN2", target_bir_lowering=False, debug=False,
                   num_devices=NCORES)

    xdev_d = nc.dram_tensor("xdev", [128, 2, 3, RQ], mt.float32,
                            kind="ExternalInput")
    xhalf_d = nc.dram_tensor("xhalf", [COUT, H * W], mt.float16,
                             kind="ExternalInput")
    wdr_d = nc.dram_tensor("wdr", [96, 5 * 2 * 128], mt.float8e4,
                           kind="ExternalInput")
    cst_d = nc.dram_tensor("cst", [64, 4], mt.float32, kind="ExternalInput")
    coef_d = nc.dram_tensor("coef", [128, 8], mt.float32, kind="ExternalInput")
    out_d = nc.dram_tensor("out", [2, COUT, NBANK, 448], mt.float32,
                           kind="ExternalOutput")

    with tile.TileContext(nc) as tc:
        with tc.tile_pool(name="main", bufs=1) as P, \
             tc.tile_pool(name="psum", bufs=6, space="PSUM") as PS, \
             tc.tile_pool(name="dram", bufs=1, space="DRAM") as D:

            # ---- constants ----
            wdr = P.tile([96, 5, 2, 128], mt.float8e4)
            nc.sync.dma_start(
                wdr[:], wdr_d.ap().rearrange("p (d t m) -> p d t m", d=5, t=2))
            cst = P.tile([64, 4], mt.float32)
            nc.sync.dma_start(cst[:], cst_d.ap())
            coef = P.tile([128, 8], mt.float32)
            if general_affine:
                nc.sync.dma_start(coef[:], coef_d.ap())

            # ---- persistent tiles ----
            xa2f = P.tile([96, PW, PW], mt.float8e4)
            y = P.tile([128, NBANK, 448], mt.float16)
            sums = P.tile([128, NBANK], mt.float32)
            sqs = P.tile([128, NBANK], mt.float32)

            neg1 = P.tile([128, 1], mt.float32)
            nc.vector.memset(neg1[:], -1.0)
            two = P.tile([128, 1], mt.float32)
            nc.vector.memset(two[:], 2.0)

            # top/bottom pads; A row 224 is rewritten by the (h1, s7) scatter
            nc.vector.memset(xa2f[0:96, 0, :], 0.0)
            nc.vector.memset(xa2f[0:96, 224:226, :], 0.0)

            # ---- binarize + scatter, chunk order (hf, c) ----
            # all loads issued first so the SP queue never head-blocks them
            x1bs = {}
            for hf in range(2):
                for c in range(3):
                    x1b = P.tile([128, RQ], mt.float32, tag="x1", bufs=2,
                                 name=f"x1b_{hf}_{c}")
                    nc.sync.dma_start(x1b[:], xdev_d.ap()[:, hf, c, :])
                    x1bs[(hf, c)] = x1b
            for hf in range(2):
                for c in range(3):
                    x1b = x1bs[(hf, c)]
                    if general_affine:
                        nc.vector.tensor_scalar(
                            x1b[:], x1b[:], coef[:, c:c + 1],
                            coef[:, 3 + c:4 + c], AO.mult, AO.add)
                    m1 = P.tile([128, RQ], mt.bfloat16, tag="m1", bufs=2,
                                name=f"m1_{hf}_{c}")
                    nc.vector.tensor_scalar(m1[:], x1b[:], MAGIC, MAGIC,
                                            AO.add, AO.subtract)
                    t2b = P.tile([128, RQ], mt.float8e4, tag="t2b", bufs=2,
                                 name=f"t2b_{hf}_{c}")
                    nc.vector.tensor_tensor(t2b[:], x1b[:], m1[:], AO.is_ge)
                    xa1b = P.tile([128, 14, PW], mt.float8e4, tag="xa1b", bufs=2,
                                  name=f"xa1b_{hf}_{c}")
                    nc.scalar.activation(
                        xa1b[:].rearrange("p a b -> p (a b)"), t2b[:],
                        AF.Identity, bias=neg1[:], scale=two[:])
                    # zero the column pads
                    nc.gpsimd.memset(xa1b[:, :, 0], 0.0)
                    nc.gpsimd.memset(xa1b[:, :, 225], 0.0)
                    # scatter all 8 segs in one DMA (partition p = 8g + s)
                    abase = xa2f[16 * c:16 * c + 16, 0:1, 0:1]
                    dst = AP(abase.tensor,
                             int(abase.offset) + (14 * hf + 1) * PW,
                             [[int(abase.ap[0][0]), 16], [28 * PW, SEGS],
                              [1, RQ]])
                    nc.sync.dma_start(dst, xa1b[:].rearrange("p a b -> p (a b)"))
                # B-half copy for this hf: B[r] = A[r+1], all segs in one DMA
                bb = xa2f[48:96, 0:1, 0:1]
                ab = xa2f[0:48, 0:1, 0:1]
                bdst = AP(bb.tensor, int(bb.offset) + 14 * hf * PW,
                          [[int(bb.ap[0][0]), 48], [28 * PW, SEGS], [1, RQ]])
                bsrc = AP(ab.tensor, int(ab.offset) + (14 * hf + 1) * PW,
                          [[int(ab.ap[0][0]), 48], [28 * PW, SEGS], [1, RQ]])
                nc.sync.dma_start(bdst, bsrc)

            # ---- conv: DoubleRow matmuls ----
            xbase = xa2f[0:96, 0:1, 0:1]
            pstride = int(xbase.ap[0][0])
            xoff = int(xbase.offset)

            # bank emission order: phase A (ready after hf0), then per-seg rest
            order = [(0, 0)] + [(s, j) for s in range(SEGS) for j in (1, 2)]
            for s in range(SEGS):
                order += [(s, 3), (s, 4), (s, 5), (s, 6)]
                if s + 1 < SEGS:
                    order.append((s + 1, 0))

            perf = mybir.MatmulPerfMode.DoubleRow
            for (s, j) in order:
                b = 7 * s + j
                y0 = 4 * b
                ps = PS.tile([128, 448], mt.float32, tag="ps", name=f"ps_{b}")
                for r in range(2):
                    for d, (t0, t1) in enumerate(DR_PAIRS):
                        o0 = (y0 + r + t0[0]) * PW + t0[1]
                        o1 = (y0 + r + t1[0]) * PW + t1[1]
                        mv = AP(xbase.tensor, xoff + o0,
                                [[pstride, 96], [o1 - o0, 2], [1, 224]])
                        nc.tensor.matmul(ps[:, 224 * r:224 * r + 224],
                                         wdr[:, d, :, :], mv,
                                         start=(d == 0), stop=(d == 4),
                                         perf_mode=perf)
                nc.vector.tensor_scalar(y[:, b, :], ps[:], 1.0, None,
                                        AO.mult, AO.add,
                                        accum_out=sums[:, b:b + 1])
                nc.scalar.activation(ps[:], ps[:], AF.Square,
                                     accum_out=sqs[:, b:b + 1])

            # ---- bypass loads (fp16, straight into y layout) ----
            byp_tiles = {}

            def load_byp(s):
                bp = P.tile([128, 7, 448], mt.float16, tag="byp", bufs=5,
                            name=f"byp_{s}")
                for ci in range(2):
                    src = AP(xhalf_d.ap().tensor, 6272 * s + 448 * ci,
                             [[H * W, COUT], [896, 7], [1, 448]])
                    nc.sync.dma_start(bp[64 * ci:64 * ci + 64, :, :], src)
                return bp

            for s in range(5):
                byp_tiles[s] = load_byp(s)

            # ---- stats + collective + BN affine ----
            kc = P.tile([128, 2], mt.float32)
            ssum = P.tile([128, 1], mt.float32)
            ssq = P.tile([128, 1], mt.float32)
            nc.vector.reduce_sum(ssum[:], sums[:], axis=mybir.AxisListType.X)
            nc.vector.reduce_sum(ssq[:], sqs[:], axis=mybir.AxisListType.X)
            toph = P.tile([64, 2], mt.float32)
            nc.scalar.dma_start(toph[:, 0:1], ssum[64:128, :])
            nc.scalar.dma_start(toph[:, 1:2], ssq[64:128, :])
            cb = P.tile([64, 2], mt.float32)
            nc.vector.tensor_tensor(cb[:, 0:1], ssum[0:64, :], toph[:, 0:1],
                                    AO.add)
            nc.vector.tensor_tensor(cb[:, 1:2], ssq[0:64, :], toph[:, 1:2],
                                    AO.add)
            nc.vector.tensor_scalar(cb[:], cb[:], 1.0 / float(B * H * W),
                                    None, AO.mult)
            cbin = D.tile([64, 2], mt.float32)
            cbout = D.tile([NCORES, 64, 2], mt.float32)
            nc.scalar.dma_start(cbin[:], cb[:])
            nc.gpsimd.collective_compute(
                "AllGather", mybir.AluOpType.bypass,
                replica_groups=[list(range(NCORES))],
                ins=[cbin.opt()], outs=[cbout.opt()])
            gath = P.tile([64, 2, NCORES], mt.float32)
            nc.scalar.dma_start(gath[:], cbout[:].rearrange("g p q -> p q g"))
            mv2 = P.tile([64, 2], mt.float32)
            nc.vector.reduce_sum(mv2[:], gath[:], axis=mybir.AxisListType.X)

            m2t = P.tile([64, 1], mt.float32)
            nc.vector.tensor_tensor(m2t[:], mv2[:, 0:1], mv2[:, 0:1], AO.mult)
            vart = P.tile([64, 1], mt.float32)
            nc.vector.tensor_tensor(vart[:], mv2[:, 1:2], m2t[:], AO.subtract)
            t1 = P.tile([64, 1], mt.float32)
            nc.vector.tensor_tensor(t1[:], vart[:], cst[:, 0:1], AO.mult)
            nc.vector.tensor_scalar(t1[:], t1[:], BN_EPS, None, AO.add)
            sq = P.tile([64, 1], mt.float32)
            nc.scalar.activation(sq[:], t1[:], AF.Sqrt)
            rc = P.tile([64, 1], mt.float32)
            nc.vector.reciprocal(rc[:], sq[:])
            nc.vector.tensor_tensor(kc[0:64, 0:1], rc[:], cst[:, 1:2], AO.mult)
            mk = P.tile([64, 1], mt.float32)
            nc.vector.tensor_tensor(mk[:], mv2[:, 0:1], kc[0:64, 0:1], AO.mult)
            nc.vector.tensor_tensor(kc[0:64, 1:2], cst[:, 2:3], mk[:],
                                    AO.subtract)
            nc.scalar.dma_start(kc[64:128, :], kc[0:64, :])

            # ---- pass 2: affine + bypass + store ----
            for s in range(SEGS):
                bp = byp_tiles.pop(s) if s in byp_tiles else load_byp(s)
                # byp += c (fp16, ACT), then ob = y*k + bypc in one DVE op
                nc.scalar.activation(bp[:], bp[:], AF.Identity,
                                     bias=kc[:, 1:2])
                ob = P.tile([128, 7, 448], mt.float32, tag="ob", bufs=2,
                            name=f"ob_{s}")
                nc.vector.scalar_tensor_tensor(ob[:], y[:, 7 * s:7 * s + 7, :],
                                               kc[:, 0:1], bp[:],
                                               AO.mult, AO.add)
                nc.gpsimd.dma_start(out_d.ap()[:, :, 7 * s:7 * s + 7, :], ob[:])

    nc.compile()
    return nc


def _get_nc(general_affine):
    key = ("nc", general_affine, NCORES)
    if key not in _cache:
        _cache[key] = _build(general_affine)
    return _cache[key]


def _pack_weights(wt):
    """wt [64, 48, 3, 3] (+-1 * A, slot-permuted) -> [96, 5, 2, 128] f32."""
    w = np.zeros((96, 5, 2, 128), np.float32)
    covered = set()
    for d, pair in enumerate(DR_PAIRS):
        for t, tl in enumerate(pair):
            if len(tl) == 3:
                continue
            delta, kw = tl
            for stack in (0, 1):
                for half, rho in ((0, 0), (1, 2)):
                    kh = delta + stack - rho
                    if 0 <= kh <= 2 and (rho, kh, kw) not in covered:
                        covered.add((rho, kh, kw))
                        w[48 * stack:48 * stack + 48, d, t,
                          64 * half:64 * half + 64] = wt[:, :, kh, kw].T
    assert len(covered) == 18
    return w


def _host_prep(alpha, epsilon, tau, A, weight, gamma, beta):
    import ml_dtypes
    f8 = ml_dtypes.float8_e4m3

    eps_v = np.asarray(epsilon, np.float32).reshape(-1)
    tau_v = np.asarray(tau, np.float32).reshape(-1)
    A_v = np.asarray(A, np.float32).reshape(-1)
    if eps_v.size == 1:
        eps_v = np.full(CIN, eps_v[0], np.float32)
    if tau_v.size == 1:
        tau_v = np.full(CIN, tau_v[0], np.float32)
    if A_v.size == 1:
        A_v = np.full(CIN, A_v[0], np.float32)

    general = not (np.all(eps_v == 0.0) and np.all(tau_v == 1.0))

    w = np.asarray(weight, np.float32)
    scale = np.mean(np.abs(w), axis=(1, 2, 3), dtype=np.float32)
    waff = np.sign(w) * A_v[None, :, None, None]
    wperm = waff[:, SLOT_TO_CH, :, :]
    wdr = _pack_weights(wperm).reshape(96, -1).astype(f8)

    cst = np.zeros((64, 4), np.float32)
    cst[:, 0] = scale * scale
    cst[:, 1] = np.asarray(gamma, np.float32).reshape(-1) * scale
    cst[:, 2] = np.asarray(beta, np.float32).reshape(-1)

    coef = np.zeros((128, 8), np.float32)
    if general:
        for p in range(128):
            g = p // 8
            for c in range(3):
                ch = 45 + c if g == 15 else 15 * c + g
                coef[p, c] = 1.0 / tau_v[ch]
                coef[p, 3 + c] = -eps_v[ch] / tau_v[ch]
    return general, wdr, cst, coef


def _make_xdev(xi):
    """xi [48, 224, 224] f32 -> [128, 2, 3, 3164] (rows padded to 226)."""
    xp = np.zeros((CIN, H, PW), np.float32)
    xp[:, :, 1:225] = xi
    xr = xp.reshape(CIN, SEGS, 2, RQ)       # [ch, seg, hf, 14*226]
    p = np.arange(128)
    g_idx, s_idx = p // 8, p % 8
    out = np.empty((128, 2, 3, RQ), np.float32)
    for c in range(3):
        ch = np.where(g_idx == 15, 45 + c, 15 * c + g_idx)
        out[:, :, c, :] = xr[ch, s_idx, :, :]
    return out


def _make_xhalf(xi):
    """xi [48, 224, 224] f32 -> [64, H*W] fp16 (identity + 16 group means)."""
    xh = np.empty((COUT, H * W), np.float16)
    xh[0:CIN] = xi.reshape(CIN, -1).astype(np.float16)
    xf = xi.reshape(CIN, -1)
    xh[48:63] = xf[0:45].reshape(3, 15, -1).mean(axis=0,
                                                 dtype=np.float32).astype(np.float16)
    xh[63] = xf[45:48].mean(axis=0, dtype=np.float32).astype(np.float16)
    return xh


def kernel(x, alpha, epsilon, tau, A, weight, gamma, beta):
    from concourse import bass_utils

    x = np.asarray(x, np.float32)
    general, wdr, cst, coef = _host_prep(alpha, epsilon, tau, A,
                                         weight, gamma, beta)
    nc = _get_nc(general)

    in_maps = []
    for i in range(NCORES):
        xi = np.ascontiguousarray(x[i])
        in_maps.append({
            "xdev": _make_xdev(xi),
            "xhalf": _make_xhalf(xi),
            "wdr": wdr, "cst": cst, "coef": coef,
        })
    res = bass_utils.run_bass_kernel_spmd(nc, in_maps,
                                          core_ids=list(range(NCORES)))
    out = np.stack([
        res.results[i]["out"].reshape(2, COUT, NBANK, 2, 224)
        .transpose(1, 2, 0, 3, 4).reshape(COUT, H, W)
        for i in range(NCORES)
    ])
    return out.astype(np.float32)


# revision 32
# speedup vs baseline: 1.9492x; 1.0835x over previous
"""Trainium2 Bass kernel for nn_BiDenseConv2d (binarized 3x3 conv + sync-BN + channel bypass).

Shapes (hardcoded): x [8, 48, 224, 224] f32 -> out [8, 64, 224, 224] f32.
Sharding: data-parallel over batch, 1 image per core; BN stats all-reduced
([64,2] f32 AllGather); weights replicated.

Per-core pipeline (phases overlap via Tile dataflow):
  1. binarize (chunks of [128p=(seg,grp), 14 rows x 226]): rint via fp32 magic
     (Pool), is_ge (DVE), affine {1,0}->+-1 fp8 (ACT); column pads zeroed.
  2. scatter to conv layout xa2f [96, 226, 226] fp8 (A=rows, B=rows+1 copy),
     one 3164B-contiguous DMA per (c, hf, seg); B-half via 16 in-tile copies.
  3. conv: fp8 DoubleRow matmuls, M=128 (low 64 partitions = out rows y0+r,
     high = y0+2+r), N=224, 10 DR per 4-row bank; taps packed via tiles
     (delta in {0,2,4}) x kw over the 2-row stack; +-1 acts x +-1 weights give
     exact integer sums in PSUM f32, evicted to fp16.
  4. BN: sums (DVE evict accum) + sumsq (ACT Square accum); AllGather [64,2];
     k = gamma*s*rsqrt(s^2 var + eps), c = beta - mu*k  (s = mean|w|).
  5. bypass: host supplies xhalf fp16 [64, H*W] (48 identity channels + 16
     3-channel means); loaded per seg straight into the y layout; pass 2:
     ob = affine(y) (ACT) + byp (DVE), stored via Pool SWDGE.

Conv input channel at slot 16c+g is channel 15c+g (g<15) / 45+c (g=15),
folded into the weights host-side. Output layout matches the baseline.
"""
import sys
import numpy as np

sys.path.insert(0, '/opt/trn_rl_repo')

B, CIN, COUT, H, W = 8, 48, 64, 224, 224
NCORES = 8
SEGS, SEGR = 8, 28
PW = 226
RQ = 14 * PW            # 3164 elems per (c, hf) chunk row-block
NBANK = 56
BN_EPS = 1e-5
MAGIC = 12582912.0

_cache = {}

SLOT_TO_CH = np.zeros(48, np.int64)
for _c in range(3):
    for _g in range(16):
        SLOT_TO_CH[16 * _c + _g] = (45 + _c) if _g == 15 else (15 * _c + _g)

# DR tile pairs (delta, kw); 'z' = zero-weight tile (arbitrary in-bounds read)
DR_PAIRS = [((0, 0), (2, 0)),
            ((0, 1), (2, 1)),
            ((0, 2), (4, 0)),
            ((2, 2), (4, 1)),
            ((1, 0, 'z'), (4, 2))]


def _build(general_affine: bool):
    from concourse import bacc, tile, mybir
    from concourse.ap import AP
    mt = mybir.dt
    AO = mybir.AluOpType
    AF = mybir.ActivationFunctionType

    nc = bacc.Bacc("TRN2", target_bir_lowering=False, debug=False,
                   num_devices=NCORES)

    xdev_d = nc.dram_tensor("xdev", [128, 2, 3, RQ], mt.float32,
                            kind="ExternalInput")
    xhalf_d = nc.dram_tensor("xhalf", [COUT, H * W], mt.float16,
                             kind="ExternalInput")
    wdr_d = nc.dram_tensor("wdr", [96, 5 * 2 * 128], mt.float8e4,
                           kind="ExternalInput")
    cst_d = nc.dram_tensor("cst", [64, 4], mt.float32, kind="ExternalInput")
    coef_d = nc.dram_tensor("coef", [128, 8], mt.float32, kind="ExternalInput")
    out_d = nc.dram_tensor("out", [2, COUT, NBANK, 448], mt.float32,
                           kind="ExternalOutput")

    with tile.TileContext(nc) as tc:
        with tc.tile_pool(name="main", bufs=1) as P, \
             tc.tile_pool(name="psum", bufs=8, space="PSUM") as PS, \
             tc.tile_pool(name="dram", bufs=1, space="DRAM") as D:

            # ---- constants ----
            wdr = P.tile([96, 5, 2, 128], mt.float8e4)
            nc.sync.dma_start(
                wdr[:], wdr_d.ap().rearrange("p (d t m) -> p d t m", d=5, t=2))
            cst = P.tile([64, 4], mt.float32)
            nc.sync.dma_start(cst[:], cst_d.ap())
            coef = P.tile([128, 8], mt.float32)
            if general_affine:
                nc.sync.dma_start(coef[:], coef_d.ap())

            # ---- persistent tiles ----
            xa2f = P.tile([96, PW, PW], mt.float8e4)
            y = P.tile([128, NBANK, 448], mt.float16)
            sums = P.tile([128, NBANK // 2], mt.float32)
            sqs = P.tile([128, NBANK // 2], mt.float32)

            neg1 = P.tile([128, 1], mt.float32)
            nc.vector.memset(neg1[:], -1.0)
            two = P.tile([128, 1], mt.float32)
            nc.vector.memset(two[:], 2.0)

            # top/bottom pads; A row 224 is rewritten by the (h1, s7) scatter
            nc.vector.memset(xa2f[0:96, 0, :], 0.0)
            nc.vector.memset(xa2f[0:96, 224:226, :], 0.0)

            # ---- binarize + scatter, half-chunks (hf, hh, c) of 7 rows ----
            # all loads issued first so the SP queue never head-blocks them
            HQ = RQ // 2            # 1582 = 7*226
            xv = xdev_d.ap().rearrange("p f c (h q) -> p f c h q", h=2)
            chunks = [(hf, hh, c) for hf in range(2) for hh in range(2)
                      for c in range(3)]
            x1s = {}

            def load_chunk(ch):
                hf, hh, c = ch
                x1h = P.tile([128, HQ], mt.float32, tag="x1", bufs=4,
                             name=f"x1_{hf}_{hh}_{c}")
                nc.sync.dma_start(x1h[:], xv[:, hf, c, hh, :])
                x1s[ch] = x1h

            for ch in chunks[:4]:
                load_chunk(ch)
            for ci, (hf, hh, c) in enumerate(chunks):
                x1h = x1s[(hf, hh, c)]
                if general_affine:
                    nc.vector.tensor_scalar(
                        x1h[:], x1h[:], coef[:, c:c + 1],
                        coef[:, 3 + c:4 + c], AO.mult, AO.add)
                m1 = P.tile([128, HQ], mt.bfloat16, tag="m1", bufs=4,
                            name=f"m1_{hf}_{hh}_{c}")
                nc.gpsimd.tensor_scalar(m1[:], x1h[:], MAGIC, MAGIC,
                                        AO.add, AO.subtract)
                nc.vector.tensor_tensor(m1[:], x1h[:], m1[:], AO.is_ge)
                xa1b = P.tile([128, 7, PW], mt.float8e4, tag="xa1b", bufs=2,
                              name=f"xa1b_{hf}_{hh}_{c}")
                if ci < 2:   # ring pads zeroed once; op3 writes interior only
                    nc.gpsimd.memset(xa1b[:, :, 0], 0.0)
                    nc.gpsimd.memset(xa1b[:, :, 225], 0.0)
                m1v = m1[:].rearrange("p (a b) -> p a b", a=7)
                nc.scalar.activation(xa1b[:, :, 1:225], m1v[:, :, 1:225],
                                     AF.Identity, bias=neg1[:], scale=two[:])
                # scatter all 8 segs in one DMA (partition p = 8g + s)
                abase = xa2f[16 * c:16 * c + 16, 0:1, 0:1]
                dst = AP(abase.tensor,
                         int(abase.offset) + (14 * hf + 7 * hh + 1) * PW,
                         [[int(abase.ap[0][0]), 16], [28 * PW, SEGS], [1, HQ]])
                nc.sync.dma_start(dst, xa1b[:].rearrange("p a b -> p (a b)"))
                if c == 2:
                    # B-half copy for (hf, hh): B[r] = A[r+1], one DMA
                    bb = xa2f[48:96, 0:1, 0:1]
                    ab = xa2f[0:48, 0:1, 0:1]
                    r0 = (14 * hf + 7 * hh) * PW
                    bdst = AP(bb.tensor, int(bb.offset) + r0,
                              [[int(bb.ap[0][0]), 48], [28 * PW, SEGS],
                               [1, HQ]])
                    bsrc = AP(ab.tensor, int(ab.offset) + r0 + PW,
                              [[int(ab.ap[0][0]), 48], [28 * PW, SEGS],
                               [1, HQ]])
                    nc.sync.dma_start(bdst, bsrc)
                if ci + 4 < len(chunks):
                    load_chunk(chunks[ci + 4])

            # ---- conv: DoubleRow matmuls ----
            xbase = xa2f[0:96, 0:1, 0:1]
            pstride = int(xbase.ap[0][0])
            xoff = int(xbase.offset)

            perf = mybir.MatmulPerfMode.DoubleRow
            for bp2 in range(NBANK // 2):
                ps = PS.tile([128, 2, 512], mt.float32, tag="ps", bufs=4,
                             name=f"ps_{bp2}")
                for half in range(2):
                    b = 2 * bp2 + half
                    y0 = 4 * b
                    for r in range(2):
                        for d, (t0, t1) in enumerate(DR_PAIRS):
                            o0 = (y0 + r + t0[0]) * PW + t0[1]
                            o1 = (y0 + r + t1[0]) * PW + t1[1]
                            mv = AP(xbase.tensor, xoff + o0,
                                    [[pstride, 96], [o1 - o0, 2], [1, 224]])
                            nc.tensor.matmul(
                                ps[:, half, 224 * r:224 * r + 224],
                                wdr[:, d, :, :], mv,
                                start=(d == 0), stop=(d == 4),
                                perf_mode=perf)
                nc.vector.tensor_scalar(y[:, 2 * bp2:2 * bp2 + 2, :],
                                        ps[:, :, 0:448],
                                        1.0, None, AO.mult, AO.add,
                                        accum_out=sums[:, bp2:bp2 + 1])
                nc.scalar.activation(ps[:, :, 0:448], ps[:, :, 0:448],
                                     AF.Square,
                                     accum_out=sqs[:, bp2:bp2 + 1])

            # ---- bypass loads (fp16, straight into y layout) ----
            byp_tiles = {}

            def load_byp(s):
                bp = P.tile([128, 7, 448], mt.float16, tag="byp", bufs=6,
                            name=f"byp_{s}")
                for ci in range(2):
                    src = AP(xhalf_d.ap().tensor, 6272 * s + 448 * ci,
                             [[H * W, COUT], [896, 7], [1, 448]])
                    nc.sync.dma_start(bp[64 * ci:64 * ci + 64, :, :], src)
                return bp

            for s in range(6):
                byp_tiles[s] = load_byp(s)

            # ---- stats + collective + BN affine ----
            kc = P.tile([128, 2], mt.float32)
            ssum = P.tile([128, 1], mt.float32)
            ssq = P.tile([128, 1], mt.float32)
            nc.vector.reduce_sum(ssum[:], sums[:], axis=mybir.AxisListType.X)
            nc.vector.reduce_sum(ssq[:], sqs[:], axis=mybir.AxisListType.X)
            toph = P.tile([64, 2], mt.float32)
            nc.scalar.dma_start(toph[:, 0:1], ssum[64:128, :])
            nc.scalar.dma_start(toph[:, 1:2], ssq[64:128, :])
            cb = P.tile([64, 2], mt.float32)
            nc.vector.tensor_tensor(cb[:, 0:1], ssum[0:64, :], toph[:, 0:1],
                                    AO.add)
            nc.vector.tensor_tensor(cb[:, 1:2], ssq[0:64, :], toph[:, 1:2],
                                    AO.add)
            nc.vector.tensor_scalar(cb[:], cb[:], 1.0 / float(B * H * W),
                                    None, AO.mult)
            cbin = D.tile([64, 2], mt.float32)
            cbout = D.tile([NCORES, 64, 2], mt.float32)
            nc.scalar.dma_start(cbin[:], cb[:])
            nc.gpsimd.collective_compute(
                "AllGather", mybir.AluOpType.bypass,
                replica_groups=[list(range(NCORES))],
                ins=[cbin.opt()], outs=[cbout.opt()])
            gath = P.tile([64, 2, NCORES], mt.float32)
            nc.scalar.dma_start(gath[:], cbout[:].rearrange("g p q -> p q g"))
            mv2 = P.tile([64, 2], mt.float32)
            nc.vector.reduce_sum(mv2[:], gath[:], axis=mybir.AxisListType.X)

            m2t = P.tile([64, 1], mt.float32)
            nc.vector.tensor_tensor(m2t[:], mv2[:, 0:1], mv2[:, 0:1], AO.mult)
            vart = P.tile([64, 1], mt.float32)
            nc.vector.tensor_tensor(vart[:], mv2[:, 1:2], m2t[:], AO.subtract)
            t1 = P.tile([64, 1], mt.float32)
            nc.vector.tensor_tensor(t1[:], vart[:], cst[:, 0:1], AO.mult)
            nc.vector.tensor_scalar(t1[:], t1[:], BN_EPS, None, AO.add)
            sq = P.tile([64, 1], mt.float32)
            nc.scalar.activation(sq[:], t1[:], AF.Sqrt)
            rc = P.tile([64, 1], mt.float32)
            nc.vector.reciprocal(rc[:], sq[:])
            nc.vector.tensor_tensor(kc[0:64, 0:1], rc[:], cst[:, 1:2], AO.mult)
            mk = P.tile([64, 1], mt.float32)
            nc.vector.tensor_tensor(mk[:], mv2[:, 0:1], kc[0:64, 0:1], AO.mult)
            nc.vector.tensor_tensor(kc[0:64, 1:2], cst[:, 2:3], mk[:],
                                    AO.subtract)
            nc.scalar.dma_start(kc[64:128, :], kc[0:64, :])

            # ---- pass 2: affine + bypass + store ----
            for s in range(SEGS):
                bp = byp_tiles.pop(s) if s in byp_tiles else load_byp(s)
                # byp += c (fp16, ACT), then ob = y*k + bypc in one DVE op
                nc.scalar.activation(bp[:], bp[:], AF.Identity,
                                     bias=kc[:, 1:2])
                ob = P.tile([128, 7, 448], mt.float32, tag="ob", bufs=2,
                            name=f"ob_{s}")
                nc.vector.scalar_tensor_tensor(ob[:], y[:, 7 * s:7 * s + 7, :],
                                               kc[:, 0:1], bp[:],
                                               AO.mult, AO.add)
                nc.gpsimd.dma_start(out_d.ap()[:, :, 7 * s:7 * s + 7, :], ob[:])

    nc.compile()
    return nc


def _get_nc(general_affine):
    key = ("nc", general_affine, NCORES)
    if key not in _cache:
        _cache[key] = _build(general_affine)
    return _cache[key]


def _pack_weights(wt):
    """wt [64, 48, 3, 3] (+-1 * A, slot-permuted) -> [96, 5, 2, 128] f32."""
    w = np.zeros((96, 5, 2, 128), np.float32)
    covered = set()
    for d, pair in enumerate(DR_PAIRS):
        for t, tl in enumerate(pair):
            if len(tl) == 3:
                continue
            delta, kw = tl
            for stack in (0, 1):
                for half, rho in ((0, 0), (1, 2)):
                    kh = delta + stack - rho
                    if 0 <= kh <= 2 and (rho, kh, kw) not in covered:
                        covered.add((rho, kh, kw))
                        w[48 * stack:48 * stack + 48, d, t,
                          64 * half:64 * half + 64] = wt[:, :, kh, kw].T
    assert len(covered) == 18
    return w


def _host_prep(alpha, epsilon, tau, A, weight, gamma, beta):
    import ml_dtypes
    f8 = ml_dtypes.float8_e4m3

    eps_v = np.asarray(epsilon, np.float32).reshape(-1)
    tau_v = np.asarray(tau, np.float32).reshape(-1)
    A_v = np.asarray(A, np.float32).reshape(-1)
    if eps_v.size == 1:
        eps_v = np.full(CIN, eps_v[0], np.float32)
    if tau_v.size == 1:
        tau_v = np.full(CIN, tau_v[0], np.float32)
    if A_v.size == 1:
        A_v = np.full(CIN, A_v[0], np.float32)

    general = not (np.all(eps_v == 0.0) and np.all(tau_v == 1.0))

    w = np.asarray(weight, np.float32)
    scale = np.mean(np.abs(w), axis=(1, 2, 3), dtype=np.float32)
    waff = np.sign(w) * A_v[None, :, None, None]
    wperm = waff[:, SLOT_TO_CH, :, :]
    wdr = _pack_weights(wperm).reshape(96, -1).astype(f8)

    cst = np.zeros((64, 4), np.float32)
    cst[:, 0] = scale * scale
    cst[:, 1] = np.asarray(gamma, np.float32).reshape(-1) * scale
    cst[:, 2] = np.asarray(beta, np.float32).reshape(-1)

    coef = np.zeros((128, 8), np.float32)
    if general:
        for p in range(128):
            g = p // 8
            for c in range(3):
                ch = 45 + c if g == 15 else 15 * c + g
                coef[p, c] = 1.0 / tau_v[ch]
                coef[p, 3 + c] = -eps_v[ch] / tau_v[ch]
    return general, wdr, cst, coef


def _make_xdev(xi):
    """xi [48, 224, 224] f32 -> [128, 2, 3, 3164] (rows padded to 226)."""
    xp = np.zeros((CIN, H, PW), np.float32)
    xp[:, :, 1:225] = xi
    xr = xp.reshape(CIN, SEGS, 2, RQ)       # [ch, seg, hf, 14*226]
    p = np.arange(128)
    g_idx, s_idx = p // 8, p % 8
    out = np.empty((128, 2, 3, RQ), np.float32)
    for c in range(3):
        ch = np.where(g_idx == 15, 45 + c, 15 * c + g_idx)
        out[:, :, c, :] = xr[ch, s_idx, :, :]
    return out


def _make_xhalf(xi):
    """xi [48, 224, 224] f32 -> [64, H*W] fp16 (identity + 16 group means)."""
    xh = np.empty((COUT, H * W), np.float16)
    xh[0:CIN] = xi.reshape(CIN, -1).astype(np.float16)
    xf = xi.reshape(CIN, -1)
    xh[48:63] = xf[0:45].reshape(3, 15, -1).mean(axis=0,
                                                 dtype=np.float32).astype(np.float16)
    xh[63] = xf[45:48].mean(axis=0, dtype=np.float32).astype(np.float16)
    return xh


def kernel(x, alpha, epsilon, tau, A, weight, gamma, beta):
    from concourse import bass_utils

    x = np.asarray(x, np.float32)
    general, wdr, cst, coef = _host_prep(alpha, epsilon, tau, A,
                                         weight, gamma, beta)
    nc = _get_nc(general)

    in_maps = []
    for i in range(NCORES):
        xi = np.ascontiguousarray(x[i])
        in_maps.append({
            "xdev": _make_xdev(xi),
            "xhalf": _make_xhalf(xi),
            "wdr": wdr, "cst": cst, "coef": coef,
        })
    res = bass_utils.run_bass_kernel_spmd(nc, in_maps,
                                          core_ids=list(range(NCORES)))
    out = np.stack([
        res.results[i]["out"].reshape(2, COUT, NBANK, 2, 224)
        .transpose(1, 2, 0, 3, 4).reshape(COUT, H, W)
        for i in range(NCORES)
    ])
    return out.astype(np.float32)


# revision 34
# speedup vs baseline: 2.0294x; 1.0411x over previous
"""Trainium2 Bass kernel for nn_BiDenseConv2d (binarized 3x3 conv + sync-BN + channel bypass).

Shapes (hardcoded): x [8, 48, 224, 224] f32 -> out [8, 64, 224, 224] f32.
Sharding: data-parallel over batch, 1 image per core; BN stats all-reduced
([64,2] f32 AllGather); weights replicated.

Per-core pipeline (phases overlap via Tile dataflow):
  1. binarize (chunks of [128p=(seg,grp), 14 rows x 226]): rint via fp32 magic
     (Pool), is_ge (DVE), affine {1,0}->+-1 fp8 (ACT); column pads zeroed.
  2. scatter to conv layout xa2f [96, 226, 226] fp8 (A=rows, B=rows+1 copy),
     one 3164B-contiguous DMA per (c, hf, seg); B-half via 16 in-tile copies.
  3. conv: fp8 DoubleRow matmuls, M=128 (low 64 partitions = out rows y0+r,
     high = y0+2+r), N=224, 10 DR per 4-row bank; taps packed via tiles
     (delta in {0,2,4}) x kw over the 2-row stack; +-1 acts x +-1 weights give
     exact integer sums in PSUM f32, evicted to fp16.
  4. BN: sums (DVE evict accum) + sumsq (ACT Square accum); AllGather [64,2];
     k = gamma*s*rsqrt(s^2 var + eps), c = beta - mu*k  (s = mean|w|).
  5. bypass: host supplies xhalf fp16 [64, H*W] (48 identity channels + 16
     3-channel means); loaded per seg straight into the y layout; pass 2:
     ob = affine(y) (ACT) + byp (DVE), stored via Pool SWDGE.

Conv input channel at slot 16c+g is channel 15c+g (g<15) / 45+c (g=15),
folded into the weights host-side. Output layout matches the baseline.
"""
import sys
import numpy as np

sys.path.insert(0, '/opt/trn_rl_repo')

B, CIN, COUT, H, W = 8, 48, 64, 224, 224
NCORES = 8
SEGS, SEGR = 8, 28
PW = 226
RQ = 14 * PW            # 3164 elems per (c, hf) chunk row-block
NBANK = 56
BN_EPS = 1e-5
MAGIC = 12582912.0

_cache = {}

SLOT_TO_CH = np.zeros(48, np.int64)
for _c in range(3):
    for _g in range(16):
        SLOT_TO_CH[16 * _c + _g] = (45 + _c) if _g == 15 else (15 * _c + _g)

# DR tile pairs (delta, kw); 'z' = zero-weight tile (arbitrary in-bounds read)
DR_PAIRS = [((0, 0), (2, 0)),
            ((0, 1), (2, 1)),
            ((0, 2), (4, 0)),
            ((2, 2), (4, 1)),
            ((1, 0, 'z'), (4, 2))]


def _build(general_affine: bool):
    from concourse import bacc, tile, mybir
    from concourse.ap import AP
    mt = mybir.dt
    AO = mybir.AluOpType
    AF = mybir.ActivationFunctionType

    nc = bacc.Bacc("TRN2", target_bir_lowering=False, debug=False,
                   num_devices=NCORES)

    xdev_d = nc.dram_tensor("xdev", [128, 2, 3, RQ], mt.float32,
                            kind="ExternalInput")
    xhalf_d = nc.dram_tensor("xhalf", [COUT, H * W], mt.float16,
                             kind="ExternalInput")
    wdr_d = nc.dram_tensor("wdr", [96, 5 * 2 * 128], mt.float8e4,
                           kind="ExternalInput")
    cst_d = nc.dram_tensor("cst", [128, 4], mt.float32, kind="ExternalInput")
    coef_d = nc.dram_tensor("coef", [128, 8], mt.float32, kind="ExternalInput")
    out_d = nc.dram_tensor("out", [2, COUT, NBANK, 448], mt.float32,
                           kind="ExternalOutput")

    with tile.TileContext(nc) as tc:
        with tc.tile_pool(name="main", bufs=1) as P, \
             tc.tile_pool(name="psum", bufs=8, space="PSUM") as PS, \
             tc.tile_pool(name="dram", bufs=1, space="DRAM") as D:

            # ---- persistent tiles ----
            xa2f = P.tile([96, PW, PW], mt.float8e4)
            y = P.tile([128, NBANK, 448], mt.float16)
            sums = P.tile([128, NBANK // 2], mt.float32)
            sqs = P.tile([128, NBANK // 2], mt.float32)

            neg1 = P.tile([128, 1], mt.float32)
            nc.vector.memset(neg1[:], -1.0)
            two = P.tile([128, 1], mt.float32)
            nc.vector.memset(two[:], 2.0)

            # top/bottom pads; A row 224 is rewritten by the (h1, s7) scatter
            nc.vector.memset(xa2f[0:96, 0, :], 0.0)
            nc.vector.memset(xa2f[0:96, 224:226, :], 0.0)

            # ---- binarize + scatter, half-chunks (hf, hh, c) of 7 rows ----
            # all loads issued first so the SP queue never head-blocks them
            HQ = RQ // 2            # 1582 = 7*226
            xv = xdev_d.ap().rearrange("p f c (h q) -> p f c h q", h=2)
            chunks = [(hf, hh, c) for hf in range(2) for hh in range(2)
                      for c in range(3)]
            x1s = {}

            def load_chunk(ch):
                hf, hh, c = ch
                x1h = P.tile([128, HQ], mt.float32, tag="x1", bufs=4,
                             name=f"x1_{hf}_{hh}_{c}")
                nc.sync.dma_start(x1h[:], xv[:, hf, c, hh, :])
                x1s[ch] = x1h

            for ch in chunks[:4]:
                load_chunk(ch)

            # constants (issued after the first x loads; needed much later)
            wdr = P.tile([96, 5, 2, 128], mt.float8e4)
            nc.sync.dma_start(
                wdr[:], wdr_d.ap().rearrange("p (d t m) -> p d t m", d=5, t=2))
            cst = P.tile([128, 4], mt.float32)
            nc.sync.dma_start(cst[:], cst_d.ap())
            coef = P.tile([128, 8], mt.float32)
            if general_affine:
                nc.sync.dma_start(coef[:], coef_d.ap())

            for ci, (hf, hh, c) in enumerate(chunks):
                x1h = x1s[(hf, hh, c)]
                if general_affine:
                    nc.vector.tensor_scalar(
                        x1h[:], x1h[:], coef[:, c:c + 1],
                        coef[:, 3 + c:4 + c], AO.mult, AO.add)
                m1 = P.tile([128, HQ], mt.bfloat16, tag="m1", bufs=4,
                            name=f"m1_{hf}_{hh}_{c}")
                rint_eng = nc.vector if ci >= 10 else nc.gpsimd
                rint_eng.tensor_scalar(m1[:], x1h[:], MAGIC, MAGIC,
                                       AO.add, AO.subtract)
                nc.vector.tensor_tensor(m1[:], x1h[:], m1[:], AO.is_ge)
                xa1b = P.tile([128, 7, PW], mt.float8e4, tag="xa1b", bufs=2,
                              name=f"xa1b_{hf}_{hh}_{c}")
                if ci < 2:   # ring pads zeroed once; op3 writes interior only
                    nc.gpsimd.memset(xa1b[:, :, 0], 0.0)
                    nc.gpsimd.memset(xa1b[:, :, 225], 0.0)
                m1v = m1[:].rearrange("p (a b) -> p a b", a=7)
                nc.scalar.activation(xa1b[:, :, 1:225], m1v[:, :, 1:225],
                                     AF.Identity, bias=neg1[:], scale=two[:])
                # scatter all 8 segs in one DMA (partition p = 8g + s)
                abase = xa2f[16 * c:16 * c + 16, 0:1, 0:1]
                dst = AP(abase.tensor,
                         int(abase.offset) + (14 * hf + 7 * hh + 1) * PW,
                         [[int(abase.ap[0][0]), 16], [28 * PW, SEGS], [1, HQ]])
                nc.sync.dma_start(dst, xa1b[:].rearrange("p a b -> p (a b)"))
                # B-half scatter: B[r] = A[r+1], same source chunk
                bbase = xa2f[48 + 16 * c:64 + 16 * c, 0:1, 0:1]
                bdst = AP(bbase.tensor,
                          int(bbase.offset) + (14 * hf + 7 * hh) * PW,
                          [[int(bbase.ap[0][0]), 16], [28 * PW, SEGS], [1, HQ]])
                nc.sync.dma_start(bdst, xa1b[:].rearrange("p a b -> p (a b)"))
                if ci + 4 < len(chunks):
                    load_chunk(chunks[ci + 4])

            # ---- conv: DoubleRow matmuls ----
            xbase = xa2f[0:96, 0:1, 0:1]
            pstride = int(xbase.ap[0][0])
            xoff = int(xbase.offset)

            perf = mybir.MatmulPerfMode.DoubleRow
            for bp2 in range(NBANK // 2):
                ps = PS.tile([128, 2, 512], mt.float32, tag="ps", bufs=4,
                             name=f"ps_{bp2}")
                for half in range(2):
                    b = 2 * bp2 + half
                    y0 = 4 * b
                    for r in range(2):
                        for d, (t0, t1) in enumerate(DR_PAIRS):
                            o0 = (y0 + r + t0[0]) * PW + t0[1]
                            o1 = (y0 + r + t1[0]) * PW + t1[1]
                            mv = AP(xbase.tensor, xoff + o0,
                                    [[pstride, 96], [o1 - o0, 2], [1, 224]])
                            nc.tensor.matmul(
                                ps[:, half, 224 * r:224 * r + 224],
                                wdr[:, d, :, :], mv,
                                start=(d == 0), stop=(d == 4),
                                perf_mode=perf)
                nc.vector.tensor_scalar(y[:, 2 * bp2:2 * bp2 + 2, :],
                                        ps[:, :, 0:448],
                                        1.0, None, AO.mult, AO.add,
                                        accum_out=sums[:, bp2:bp2 + 1])
                nc.scalar.activation(ps[:, :, 0:448], ps[:, :, 0:448],
                                     AF.Square,
                                     accum_out=sqs[:, bp2:bp2 + 1])

            # ---- bypass loads (fp16, straight into y layout) ----
            byp_tiles = {}

            def load_byp(s):
                bp = P.tile([128, 7, 448], mt.float16, tag="byp", bufs=6,
                            name=f"byp_{s}")
                for ci in range(2):
                    src = AP(xhalf_d.ap().tensor, 6272 * s + 448 * ci,
                             [[H * W, COUT], [896, 7], [1, 448]])
                    nc.sync.dma_start(bp[64 * ci:64 * ci + 64, :, :], src)
                return bp

            for s in range(6):
                byp_tiles[s] = load_byp(s)

            # ---- stats + collective + BN affine (all on 128 partitions) ----
            kc = P.tile([128, 2], mt.float32)
            sums2 = P.tile([128, 2], mt.float32)
            nc.vector.reduce_sum(sums2[:, 0:1], sums[:], axis=mybir.AxisListType.X)
            nc.vector.reduce_sum(sums2[:, 1:2], sqs[:], axis=mybir.AxisListType.X)
            cbin = D.tile([128, 2], mt.float32)
            cbout = D.tile([NCORES, 128, 2], mt.float32)
            nc.scalar.dma_start(cbin[:], sums2[:])
            nc.gpsimd.collective_compute(
                "AllGather", mybir.AluOpType.bypass,
                replica_groups=[list(range(NCORES))],
                ins=[cbin.opt()], outs=[cbout.opt()])
            # gather (core, half) entries onto BOTH partition halves
            gath = P.tile([128, 2, 2 * NCORES], mt.float32)
            cbt = cbout[:].rearrange("g (h p) q -> g h p q", h=2)
            for half in range(2):
                src = AP(cbt.tensor, 0,
                         [[2, 64], [1, 2], [128, 2 * NCORES]])
                nc.scalar.dma_start(gath[64 * half:64 * half + 64, :, :], src)
            mv2 = P.tile([128, 2], mt.float32)
            nc.vector.reduce_sum(mv2[:], gath[:], axis=mybir.AxisListType.X)
            nc.vector.tensor_scalar(mv2[:], mv2[:], 1.0 / float(B * H * W),
                                    None, AO.mult)

            m2t = P.tile([128, 1], mt.float32)
            nc.vector.tensor_tensor(m2t[:], mv2[:, 0:1], mv2[:, 0:1], AO.mult)
            vart = P.tile([128, 1], mt.float32)
            nc.vector.tensor_tensor(vart[:], mv2[:, 1:2], m2t[:], AO.subtract)
            t1 = P.tile([128, 1], mt.float32)
            nc.vector.tensor_tensor(t1[:], vart[:], cst[:, 0:1], AO.mult)
            nc.vector.tensor_scalar(t1[:], t1[:], BN_EPS, None, AO.add)
            sq = P.tile([128, 1], mt.float32)
            nc.scalar.activation(sq[:], t1[:], AF.Sqrt)
            rc = P.tile([128, 1], mt.float32)
            nc.vector.reciprocal(rc[:], sq[:])
            nc.vector.tensor_tensor(kc[:, 0:1], rc[:], cst[:, 1:2], AO.mult)
            mk = P.tile([128, 1], mt.float32)
            nc.vector.tensor_tensor(mk[:], mv2[:, 0:1], kc[:, 0:1], AO.mult)
            nc.vector.tensor_tensor(kc[:, 1:2], cst[:, 2:3], mk[:],
                                    AO.subtract)

            # ---- pass 2: affine + bypass + store ----
            bps = {}
            for s in range(SEGS):
                if s in byp_tiles:
                    bp = byp_tiles.pop(s)
                    nc.scalar.activation(bp[:], bp[:], AF.Identity,
                                         bias=kc[:, 1:2])
                    bps[s] = bp
            for s in range(SEGS):
                if s in bps:
                    bp = bps.pop(s)
                else:
                    bp = load_byp(s)
                    nc.scalar.activation(bp[:], bp[:], AF.Identity,
                                         bias=kc[:, 1:2])
                ob = P.tile([128, 7, 448], mt.float32, tag="ob", bufs=2,
                            name=f"ob_{s}")
                nc.vector.scalar_tensor_tensor(ob[:], y[:, 7 * s:7 * s + 7, :],
                                               kc[:, 0:1], bp[:],
                                               AO.mult, AO.add)
                nc.gpsimd.dma_start(out_d.ap()[:, :, 7 * s:7 * s + 7, :], ob[:])

    nc.compile()
    return nc


def _get_nc(general_affine):
    key = ("nc", general_affine, NCORES)
    if key not in _cache:
        _cache[key] = _build(general_affine)
    return _cache[key]


def _pack_weights(wt):
    """wt [64, 48, 3, 3] (+-1 * A, slot-permuted) -> [96, 5, 2, 128] f32."""
    w = np.zeros((96, 5, 2, 128), np.float32)
    covered = set()
    for d, pair in enumerate(DR_PAIRS):
        for t, tl in enumerate(pair):
            if len(tl) == 3:
                continue
            delta, kw = tl
            for stack in (0, 1):
                for half, rho in ((0, 0), (1, 2)):
                    kh = delta + stack - rho
                    if 0 <= kh <= 2 and (rho, kh, kw) not in covered:
                        covered.add((rho, kh, kw))
                        w[48 * stack:48 * stack + 48, d, t,
                          64 * half:64 * half + 64] = wt[:, :, kh, kw].T
    assert len(covered) == 18
    return w


def _host_prep(alpha, epsilon, tau, A, weight, gamma, beta):
    import ml_dtypes
    f8 = ml_dtypes.float8_e4m3

    eps_v = np.asarray(epsilon, np.float32).reshape(-1)
    tau_v = np.asarray(tau, np.float32).reshape(-1)
    A_v = np.asarray(A, np.float32).reshape(-1)
    if eps_v.size == 1:
        eps_v = np.full(CIN, eps_v[0], np.float32)
    if tau_v.size == 1:
        tau_v = np.full(CIN, tau_v[0], np.float32)
    if A_v.size == 1:
        A_v = np.full(CIN, A_v[0], np.float32)

    general = not (np.all(eps_v == 0.0) and np.all(tau_v == 1.0))

    w = np.asarray(weight, np.float32)
    scale = np.mean(np.abs(w), axis=(1, 2, 3), dtype=np.float32)
    waff = np.sign(w) * A_v[None, :, None, None]
    wperm = waff[:, SLOT_TO_CH, :, :]
    wdr = _pack_weights(wperm).reshape(96, -1).astype(f8)

    cst = np.zeros((64, 4), np.float32)
    cst[:, 0] = scale * scale
    cst[:, 1] = np.asarray(gamma, np.float32).reshape(-1) * scale
    cst[:, 2] = np.asarray(beta, np.float32).reshape(-1)
    cst = np.tile(cst, (2, 1))

    coef = np.zeros((128, 8), np.float32)
    if general:
        for p in range(128):
            g = p // 8
            for c in range(3):
                ch = 45 + c if g == 15 else 15 * c + g
                coef[p, c] = 1.0 / tau_v[ch]
                coef[p, 3 + c] = -eps_v[ch] / tau_v[ch]
    return general, wdr, cst, coef


def _make_xdev(xi):
    """xi [48, 224, 224] f32 -> [128, 2, 3, 3164] (rows padded to 226)."""
    xp = np.zeros((CIN, H, PW), np.float32)
    xp[:, :, 1:225] = xi
    xr = xp.reshape(CIN, SEGS, 2, RQ)       # [ch, seg, hf, 14*226]
    p = np.arange(128)
    g_idx, s_idx = p // 8, p % 8
    out = np.empty((128, 2, 3, RQ), np.float32)
    for c in range(3):
        ch = np.where(g_idx == 15, 45 + c, 15 * c + g_idx)
        out[:, :, c, :] = xr[ch, s_idx, :, :]
    return out


def _make_xhalf(xi):
    """xi [48, 224, 224] f32 -> [64, H*W] fp16 (identity + 16 group means)."""
    xh = np.empty((COUT, H * W), np.float16)
    xh[0:CIN] = xi.reshape(CIN, -1).astype(np.float16)
    xf = xi.reshape(CIN, -1)
    xh[48:63] = xf[0:45].reshape(3, 15, -1).mean(axis=0,
                                                 dtype=np.float32).astype(np.float16)
    xh[63] = xf[45:48].mean(axis=0, dtype=np.float32).astype(np.float16)
    return xh


def kernel(x, alpha, epsilon, tau, A, weight, gamma, beta):
    from concourse import bass_utils

    x = np.asarray(x, np.float32)
    general, wdr, cst, coef = _host_prep(alpha, epsilon, tau, A,
                                         weight, gamma, beta)
    nc = _get_nc(general)

    in_maps = []
    for i in range(NCORES):
        xi = np.ascontiguousarray(x[i])
        in_maps.append({
            "xdev": _make_xdev(xi),
            "xhalf": _make_xhalf(xi),
            "wdr": wdr, "cst": cst, "coef": coef,
        })
    res = bass_utils.run_bass_kernel_spmd(nc, in_maps,
                                          core_ids=list(range(NCORES)))
    out = np.stack([
        res.results[i]["out"].reshape(2, COUT, NBANK, 2, 224)
        .transpose(1, 2, 0, 3, 4).reshape(COUT, H, W)
        for i in range(NCORES)
    ])
    return out.astype(np.float32)


# revision 37
# speedup vs baseline: 2.1399x; 1.0544x over previous
"""Trainium2 Bass kernel for nn_BiDenseConv2d (binarized 3x3 conv + sync-BN + channel bypass).

Shapes (hardcoded): x [8, 48, 224, 224] f32 -> out [8, 64, 224, 224] f32.
Sharding: data-parallel over batch, 1 image per core; BN stats all-reduced
([64,2] f32 AllGather); weights replicated.

Per-core pipeline (phases overlap via Tile dataflow):
  1. binarize (chunks of [128p=(seg,grp), 14 rows x 226]): rint via fp32 magic
     (Pool), is_ge (DVE), affine {1,0}->+-1 fp8 (ACT); column pads zeroed.
  2. scatter to conv layout xa2f [96, 226, 226] fp8 (A=rows, B=rows+1 copy),
     one 3164B-contiguous DMA per (c, hf, seg); B-half via 16 in-tile copies.
  3. conv: fp8 DoubleRow matmuls, M=128 (low 64 partitions = out rows y0+r,
     high = y0+2+r), N=224, 10 DR per 4-row bank; taps packed via tiles
     (delta in {0,2,4}) x kw over the 2-row stack; +-1 acts x +-1 weights give
     exact integer sums in PSUM f32, evicted to fp16.
  4. BN: sums (DVE evict accum) + sumsq (ACT Square accum); AllGather [64,2];
     k = gamma*s*rsqrt(s^2 var + eps), c = beta - mu*k  (s = mean|w|).
  5. bypass: host supplies xhalf fp16 [64, H*W] (48 identity channels + 16
     3-channel means); loaded per seg straight into the y layout; pass 2:
     ob = affine(y) (ACT) + byp (DVE), stored via Pool SWDGE.

Conv input channel at slot 16c+g is channel 15c+g (g<15) / 45+c (g=15),
folded into the weights host-side. Output layout matches the baseline.
"""
import sys
import numpy as np

sys.path.insert(0, '/opt/trn_rl_repo')

B, CIN, COUT, H, W = 8, 48, 64, 224, 224
NCORES = 8
SEGS, SEGR = 8, 28
PW = 226
RQ = 14 * PW            # 3164 elems per (c, hf) chunk row-block
NBANK = 56
BN_EPS = 1e-5
MAGIC = 12582912.0

_cache = {}

SLOT_TO_CH = np.zeros(48, np.int64)
for _c in range(3):
    for _g in range(16):
        SLOT_TO_CH[16 * _c + _g] = (45 + _c) if _g == 15 else (15 * _c + _g)

# DR tile pairs (delta, kw); 'z' = zero-weight tile (arbitrary in-bounds read)
DR_PAIRS = [((0, 0), (2, 0)),
            ((0, 1), (2, 1)),
            ((0, 2), (4, 0)),
            ((2, 2), (4, 1)),
            ((1, 0, 'z'), (4, 2))]


def _build(general_affine: bool):
    from concourse import bacc, tile, mybir
    from concourse.ap import AP
    mt = mybir.dt
    AO = mybir.AluOpType
    AF = mybir.ActivationFunctionType

    nc = bacc.Bacc("TRN2", target_bir_lowering=False, debug=False,
                   num_devices=NCORES)

    xdev_d = nc.dram_tensor("xdev", [128, 2, 3, RQ], mt.float32,
                            kind="ExternalInput")
    xhalf_d = nc.dram_tensor("xhalf", [COUT, H * W], mt.float16,
                             kind="ExternalInput")
    wdr_d = nc.dram_tensor("wdr", [96, 5 * 2 * 128], mt.float8e4,
                           kind="ExternalInput")
    cst_d = nc.dram_tensor("cst", [128, 4], mt.float32, kind="ExternalInput")
    coef_d = nc.dram_tensor("coef", [128, 8], mt.float32, kind="ExternalInput")
    out_d = nc.dram_tensor("out", [2, COUT, NBANK, 448], mt.float32,
                           kind="ExternalOutput")

    with tile.TileContext(nc) as tc:
        with tc.tile_pool(name="main", bufs=1) as P, \
             tc.tile_pool(name="psum", bufs=8, space="PSUM") as PS, \
             tc.tile_pool(name="dram", bufs=1, space="DRAM") as D:

            # ---- persistent tiles ----
            xa2f = P.tile([96, PW, PW], mt.float8e4)
            y = P.tile([128, NBANK, 448], mt.float16)
            sums = P.tile([128, NBANK // 2], mt.float32)
            sqs = P.tile([128, NBANK // 2], mt.float32)

            neg1 = P.tile([128, 1], mt.float32)
            nc.vector.memset(neg1[:], -1.0)
            scr1 = P.tile([128, 1], mt.float32)
            nc.vector.memset(scr1[:], 1.0)
            nc.scalar.activation(scr1[:], scr1[:], AF.Sqrt)
            two = P.tile([128, 1], mt.float32)
            nc.vector.memset(two[:], 2.0)

            # top/bottom pads; A row 224 is rewritten by the (h1, s7) scatter
            nc.vector.memset(xa2f[0:96, 0, :], 0.0)
            nc.vector.memset(xa2f[0:96, 224:226, :], 0.0)

            # ---- binarize + scatter, half-chunks (hf, hh, c) of 7 rows ----
            # all loads issued first so the SP queue never head-blocks them
            HQ = RQ // 2            # 1582 = 7*226
            xv = xdev_d.ap().rearrange("p f c (h q) -> p f c h q", h=2)
            chunks = [(hf, hh, c) for hf in range(2) for hh in range(2)
                      for c in range(3)]
            x1s = {}

            def load_chunk(ch):
                hf, hh, c = ch
                x1h = P.tile([128, HQ], mt.float32, tag="x1", bufs=3,
                             name=f"x1_{hf}_{hh}_{c}")
                nc.sync.dma_start(x1h[:], xv[:, hf, c, hh, :])
                x1s[ch] = x1h

            for ch in chunks[:4]:
                load_chunk(ch)

            # constants (issued after the first x loads; needed much later)
            wdr = P.tile([96, 5, 2, 128], mt.float8e4)
            nc.sync.dma_start(
                wdr[:], wdr_d.ap().rearrange("p (d t m) -> p d t m", d=5, t=2))
            cst = P.tile([128, 4], mt.float32)
            nc.sync.dma_start(cst[:], cst_d.ap())
            coef = P.tile([128, 8], mt.float32)
            if general_affine:
                nc.sync.dma_start(coef[:], coef_d.ap())

            for ci, (hf, hh, c) in enumerate(chunks):
                x1h = x1s[(hf, hh, c)]
                if general_affine:
                    nc.vector.tensor_scalar(
                        x1h[:], x1h[:], coef[:, c:c + 1],
                        coef[:, 3 + c:4 + c], AO.mult, AO.add)
                m1 = P.tile([128, HQ], mt.bfloat16, tag="m1", bufs=3,
                            name=f"m1_{hf}_{hh}_{c}")
                rint_eng = nc.vector if ci >= 10 else nc.gpsimd
                rint_eng.tensor_scalar(m1[:], x1h[:], MAGIC, MAGIC,
                                       AO.add, AO.subtract)
                nc.vector.tensor_tensor(m1[:], x1h[:], m1[:], AO.is_ge)
                xa1b = P.tile([128, 7, PW], mt.float8e4, tag="xa1b", bufs=2,
                              name=f"xa1b_{hf}_{hh}_{c}")
                if ci < 2:   # ring pads zeroed once; op3 writes interior only
                    nc.gpsimd.memset(xa1b[:, :, 0], 0.0)
                    nc.gpsimd.memset(xa1b[:, :, 225], 0.0)
                m1v = m1[:].rearrange("p (a b) -> p a b", a=7)
                nc.scalar.activation(xa1b[:, :, 1:225], m1v[:, :, 1:225],
                                     AF.Identity, bias=neg1[:], scale=two[:])
                # scatter all 8 segs in one DMA (partition p = 8g + s)
                abase = xa2f[16 * c:16 * c + 16, 0:1, 0:1]
                dst = AP(abase.tensor,
                         int(abase.offset) + (14 * hf + 7 * hh + 1) * PW,
                         [[int(abase.ap[0][0]), 16], [28 * PW, SEGS], [1, HQ]])
                nc.sync.dma_start(dst, xa1b[:].rearrange("p a b -> p (a b)"))
                # B-half scatter: B[r] = A[r+1], same source chunk
                bbase = xa2f[48 + 16 * c:64 + 16 * c, 0:1, 0:1]
                bdst = AP(bbase.tensor,
                          int(bbase.offset) + (14 * hf + 7 * hh) * PW,
                          [[int(bbase.ap[0][0]), 16], [28 * PW, SEGS], [1, HQ]])
                nc.sync.dma_start(bdst, xa1b[:].rearrange("p a b -> p (a b)"))
                if ci + 4 < len(chunks):
                    load_chunk(chunks[ci + 4])

            # ---- conv: DoubleRow matmuls ----
            xbase = xa2f[0:96, 0:1, 0:1]
            pstride = int(xbase.ap[0][0])
            xoff = int(xbase.offset)

            perf = mybir.MatmulPerfMode.DoubleRow
            for bp2 in range(NBANK // 2):
                ps = PS.tile([128, 2, 512], mt.float32, tag="ps", bufs=4,
                             name=f"ps_{bp2}")
                for half in range(2):
                    b = 2 * bp2 + half
                    y0 = 4 * b
                    for r in range(2):
                        for d, (t0, t1) in enumerate(DR_PAIRS):
                            o0 = (y0 + r + t0[0]) * PW + t0[1]
                            o1 = (y0 + r + t1[0]) * PW + t1[1]
                            mv = AP(xbase.tensor, xoff + o0,
                                    [[pstride, 96], [o1 - o0, 2], [1, 224]])
                            nc.tensor.matmul(
                                ps[:, half, 224 * r:224 * r + 224],
                                wdr[:, d, :, :], mv,
                                start=(d == 0), stop=(d == 4),
                                perf_mode=perf)
                nc.vector.tensor_scalar(y[:, 2 * bp2:2 * bp2 + 2, :],
                                        ps[:, :, 0:448],
                                        1.0, None, AO.mult, AO.add,
                                        accum_out=sums[:, bp2:bp2 + 1])
                nc.scalar.activation(ps[:, :, 0:448], ps[:, :, 0:448],
                                     AF.Square,
                                     accum_out=sqs[:, bp2:bp2 + 1])

            # ---- bypass loads (fp16, straight into y layout) ----
            byp_tiles = {}

            def load_byp(s):
                bp = P.tile([128, 7, 448], mt.float16, tag="byp", bufs=8,
                            name=f"byp_{s}")
                for ci in range(2):
                    src = AP(xhalf_d.ap().tensor, 6272 * s + 448 * ci,
                             [[H * W, COUT], [896, 7], [1, 448]])
                    nc.sync.dma_start(bp[64 * ci:64 * ci + 64, :, :], src)
                return bp

            for s in range(SEGS):
                byp_tiles[s] = load_byp(s)

            # ---- stats + collective + BN affine (all on 128 partitions) ----
            kc = P.tile([128, 2], mt.float32)
            sums2 = P.tile([128, 2], mt.float32)
            nc.vector.reduce_sum(sums2[:, 0:1], sums[:], axis=mybir.AxisListType.X)
            nc.vector.reduce_sum(sums2[:, 1:2], sqs[:], axis=mybir.AxisListType.X)
            cbin = D.tile([128, 2], mt.float32)
            cbout = D.tile([NCORES, 128, 2], mt.float32)
            nc.scalar.dma_start(cbin[:], sums2[:])
            nc.gpsimd.collective_compute(
                "AllGather", mybir.AluOpType.bypass,
                replica_groups=[list(range(NCORES))],
                ins=[cbin.opt()], outs=[cbout.opt()])
            # gather (core, half) entries onto BOTH partition halves
            gath = P.tile([128, 2, 2 * NCORES], mt.float32)
            cbt = cbout[:].rearrange("g (h p) q -> g h p q", h=2)
            for half in range(2):
                src = AP(cbt.tensor, 0,
                         [[2, 64], [1, 2], [128, 2 * NCORES]])
                nc.scalar.dma_start(gath[64 * half:64 * half + 64, :, :], src)
            mv2 = P.tile([128, 2], mt.float32)
            nc.vector.reduce_sum(mv2[:], gath[:], axis=mybir.AxisListType.X)
            nc.vector.tensor_scalar(mv2[:], mv2[:], 1.0 / float(B * H * W),
                                    None, AO.mult)

            m2t = P.tile([128, 1], mt.float32)
            nc.vector.tensor_tensor(m2t[:], mv2[:, 0:1], mv2[:, 0:1], AO.mult)
            vart = P.tile([128, 1], mt.float32)
            nc.vector.tensor_tensor(vart[:], mv2[:, 1:2], m2t[:], AO.subtract)
            t1 = P.tile([128, 1], mt.float32)
            nc.vector.tensor_tensor(t1[:], vart[:], cst[:, 0:1], AO.mult)
            nc.vector.tensor_scalar(t1[:], t1[:], BN_EPS, None, AO.add)
            sq = P.tile([128, 1], mt.float32)
            nc.scalar.activation(sq[:], t1[:], AF.Sqrt)
            rc = P.tile([128, 1], mt.float32)
            nc.vector.reciprocal(rc[:], sq[:])
            nc.vector.tensor_tensor(kc[:, 0:1], rc[:], cst[:, 1:2], AO.mult)
            mk = P.tile([128, 1], mt.float32)
            nc.vector.tensor_tensor(mk[:], mv2[:, 0:1], kc[:, 0:1], AO.mult)
            nc.vector.tensor_tensor(kc[:, 1:2], cst[:, 2:3], mk[:],
                                    AO.subtract)

            # ---- pass 2: affine + bypass + store ----
            for s in range(SEGS):
                bp = byp_tiles.pop(s)
                nc.scalar.activation(bp[:], bp[:], AF.Identity,
                                     bias=kc[:, 1:2])
                for (j0, nj) in ((0, 4), (4, 3)):
                    ob = P.tile([128, 4, 448], mt.float32, tag="ob", bufs=3,
                                name=f"ob_{s}_{j0}")
                    nc.vector.scalar_tensor_tensor(
                        ob[:, 0:nj, :], y[:, 7 * s + j0:7 * s + j0 + nj, :],
                        kc[:, 0:1], bp[:, j0:j0 + nj, :], AO.mult, AO.add)
                    nc.gpsimd.dma_start(
                        out_d.ap()[:, :, 7 * s + j0:7 * s + j0 + nj, :],
                        ob[:, 0:nj, :])

    nc.compile()
    return nc


def _get_nc(general_affine):
    key = ("nc", general_affine, NCORES)
    if key not in _cache:
        _cache[key] = _build(general_affine)
    return _cache[key]


def _pack_weights(wt):
    """wt [64, 48, 3, 3] (+-1 * A, slot-permuted) -> [96, 5, 2, 128] f32."""
    w = np.zeros((96, 5, 2, 128), np.float32)
    covered = set()
    for d, pair in enumerate(DR_PAIRS):
        for t, tl in enumerate(pair):
            if len(tl) == 3:
                continue
            delta, kw = tl
            for stack in (0, 1):
                for half, rho in ((0, 0), (1, 2)):
                    kh = delta + stack - rho
                    if 0 <= kh <= 2 and (rho, kh, kw) not in covered:
                        covered.add((rho, kh, kw))
                        w[48 * stack:48 * stack + 48, d, t,
                          64 * half:64 * half + 64] = wt[:, :, kh, kw].T
    assert len(covered) == 18
    return w


def _host_prep(alpha, epsilon, tau, A, weight, gamma, beta):
    import ml_dtypes
    f8 = ml_dtypes.float8_e4m3

    eps_v = np.asarray(epsilon, np.float32).reshape(-1)
    tau_v = np.asarray(tau, np.float32).reshape(-1)
    A_v = np.asarray(A, np.float32).reshape(-1)
    if eps_v.size == 1:
        eps_v = np.full(CIN, eps_v[0], np.float32)
    if tau_v.size == 1:
        tau_v = np.full(CIN, tau_v[0], np.float32)
    if A_v.size == 1:
        A_v = np.full(CIN, A_v[0], np.float32)

    general = not (np.all(eps_v == 0.0) and np.all(tau_v == 1.0))

    w = np.asarray(weight, np.float32)
    scale = np.mean(np.abs(w), axis=(1, 2, 3), dtype=np.float32)
    waff = np.sign(w) * A_v[None, :, None, None]
    wperm = waff[:, SLOT_TO_CH, :, :]
    wdr = _pack_weights(wperm).reshape(96, -1).astype(f8)

    cst = np.zeros((64, 4), np.float32)
    cst[:, 0] = scale * scale
    cst[:, 1] = np.asarray(gamma, np.float32).reshape(-1) * scale
    cst[:, 2] = np.asarray(beta, np.float32).reshape(-1)
    cst = np.tile(cst, (2, 1))

    coef = np.zeros((128, 8), np.float32)
    if general:
        for p in range(128):
            g = p // 8
            for c in range(3):
                ch = 45 + c if g == 15 else 15 * c + g
                coef[p, c] = 1.0 / tau_v[ch]
                coef[p, 3 + c] = -eps_v[ch] / tau_v[ch]
    return general, wdr, cst, coef


def _make_xdev(xi):
    """xi [48, 224, 224] f32 -> [128, 2, 3, 3164] (rows padded to 226)."""
    xp = np.zeros((CIN, H, PW), np.float32)
    xp[:, :, 1:225] = xi
    xr = xp.reshape(CIN, SEGS, 2, RQ)       # [ch, seg, hf, 14*226]
    p = np.arange(128)
    g_idx, s_idx = p // 8, p % 8
    out = np.empty((128, 2, 3, RQ), np.float32)
    for c in range(3):
        ch = np.where(g_idx == 15, 45 + c, 15 * c + g_idx)
        out[:, :, c, :] = xr[ch, s_idx, :, :]
    return out


def _make_xhalf(xi):
    """xi [48, 224, 224] f32 -> [64, H*W] fp16 (identity + 16 group means)."""
    xh = np.empty((COUT, H * W), np.float16)
    xh[0:CIN] = xi.reshape(CIN, -1).astype(np.float16)
    xf = xi.reshape(CIN, -1)
    xh[48:63] = xf[0:45].reshape(3, 15, -1).mean(axis=0,
                                                 dtype=np.float32).astype(np.float16)
    xh[63] = xf[45:48].mean(axis=0, dtype=np.float32).astype(np.float16)
    return xh


def kernel(x, alpha, epsilon, tau, A, weight, gamma, beta):
    from concourse import bass_utils

    x = np.asarray(x, np.float32)
    general, wdr, cst, coef = _host_prep(alpha, epsilon, tau, A,
                                         weight, gamma, beta)
    nc = _get_nc(general)

    in_maps = []
    for i in range(NCORES):
        xi = np.ascontiguousarray(x[i])
        in_maps.append({
            "xdev": _make_xdev(xi),
            "xhalf": _make_xhalf(xi),
            "wdr": wdr, "cst": cst, "coef": coef,
        })
    res = bass_utils.run_bass_kernel_spmd(nc, in_maps,
                                          core_ids=list(range(NCORES)))
    out = np.stack([
        res.results[i]["out"].reshape(2, COUT, NBANK, 2, 224)
        .transpose(1, 2, 0, 3, 4).reshape(COUT, H, W)
        for i in range(NCORES)
    ])
    return out.astype(np.float32)


# revision 39
# speedup vs baseline: 2.1565x; 1.0078x over previous
"""Trainium2 Bass kernel for nn_BiDenseConv2d (binarized 3x3 conv + sync-BN + channel bypass).

Shapes (hardcoded): x [8, 48, 224, 224] f32 -> out [8, 64, 224, 224] f32.
Sharding: data-parallel over batch, 1 image per core; BN stats all-reduced
([64,2] f32 AllGather); weights replicated.

Per-core pipeline (phases overlap via Tile dataflow):
  1. binarize (chunks of [128p=(seg,grp), 14 rows x 226]): rint via fp32 magic
     (Pool), is_ge (DVE), affine {1,0}->+-1 fp8 (ACT); column pads zeroed.
  2. scatter to conv layout xa2f [96, 226, 226] fp8 (A=rows, B=rows+1 copy),
     one 3164B-contiguous DMA per (c, hf, seg); B-half via 16 in-tile copies.
  3. conv: fp8 DoubleRow matmuls, M=128 (low 64 partitions = out rows y0+r,
     high = y0+2+r), N=224, 10 DR per 4-row bank; taps packed via tiles
     (delta in {0,2,4}) x kw over the 2-row stack; +-1 acts x +-1 weights give
     exact integer sums in PSUM f32, evicted to fp16.
  4. BN: sums (DVE evict accum) + sumsq (ACT Square accum); AllGather [64,2];
     k = gamma*s*rsqrt(s^2 var + eps), c = beta - mu*k  (s = mean|w|).
  5. bypass: host supplies xhalf fp16 [64, H*W] (48 identity channels + 16
     3-channel means); loaded per seg straight into the y layout; pass 2:
     ob = affine(y) (ACT) + byp (DVE), stored via Pool SWDGE.

Conv input channel at slot 16c+g is channel 15c+g (g<15) / 45+c (g=15),
folded into the weights host-side. Output layout matches the baseline.
"""
import sys
import numpy as np

sys.path.insert(0, '/opt/trn_rl_repo')

B, CIN, COUT, H, W = 8, 48, 64, 224, 224
NCORES = 8
SEGS, SEGR = 8, 28
PW = 226
RQ = 14 * PW            # 3164 elems per (c, hf) chunk row-block
NBANK = 56
BN_EPS = 1e-5
MAGIC = 12582912.0

_cache = {}

SLOT_TO_CH = np.zeros(48, np.int64)
for _c in range(3):
    for _g in range(16):
        SLOT_TO_CH[16 * _c + _g] = (45 + _c) if _g == 15 else (15 * _c + _g)

# DR tile pairs (delta, kw); 'z' = zero-weight tile (arbitrary in-bounds read)
DR_PAIRS = [((0, 0), (2, 0)),
            ((0, 1), (2, 1)),
            ((0, 2), (4, 0)),
            ((2, 2), (4, 1)),
            ((1, 0, 'z'), (4, 2))]


def _build(general_affine: bool):
    from concourse import bacc, tile, mybir
    from concourse.ap import AP
    mt = mybir.dt
    AO = mybir.AluOpType
    AF = mybir.ActivationFunctionType

    nc = bacc.Bacc("TRN2", target_bir_lowering=False, debug=False,
                   num_devices=NCORES)

    xdev_d = nc.dram_tensor("xdev", [128, 2, 3, RQ], mt.float32,
                            kind="ExternalInput")
    xhalf_d = nc.dram_tensor("xhalf", [COUT, H * W], mt.float16,
                             kind="ExternalInput")
    wdr_d = nc.dram_tensor("wdr", [96, 5 * 2 * 128], mt.float8e4,
                           kind="ExternalInput")
    cst_d = nc.dram_tensor("cst", [128, 4], mt.float32, kind="ExternalInput")
    coef_d = nc.dram_tensor("coef", [128, 8], mt.float32, kind="ExternalInput")
    out_d = nc.dram_tensor("out", [2, COUT, NBANK, 448], mt.float32,
                           kind="ExternalOutput")

    with tile.TileContext(nc) as tc:
        with tc.tile_pool(name="main", bufs=1) as P, \
             tc.tile_pool(name="psum", bufs=8, space="PSUM") as PS, \
             tc.tile_pool(name="dram", bufs=1, space="DRAM") as D:

            # ---- persistent tiles ----
            xa2f = P.tile([96, PW, PW], mt.float8e4)
            y = P.tile([128, NBANK, 448], mt.float16)
            sums = P.tile([128, NBANK // 2], mt.float32)
            sqs = P.tile([128, NBANK // 2], mt.float32)

            neg1 = P.tile([128, 1], mt.float32)
            nc.vector.memset(neg1[:], -1.0)
            scr1 = P.tile([128, 1], mt.float32)
            nc.vector.memset(scr1[:], 1.0)
            nc.scalar.activation(scr1[:], scr1[:], AF.Sqrt)
            two = P.tile([128, 1], mt.float32)
            nc.vector.memset(two[:], 2.0)

            # top/bottom pads; A row 224 is rewritten by the (h1, s7) scatter
            nc.vector.memset(xa2f[0:96, 0, :], 0.0)
            nc.vector.memset(xa2f[0:96, 224:226, :], 0.0)

            # ---- binarize + scatter, half-chunks (hf, hh, c) of 7 rows ----
            # all loads issued first so the SP queue never head-blocks them
            HQ = RQ // 2            # 1582 = 7*226
            xv = xdev_d.ap().rearrange("p f c (h q) -> p f c h q", h=2)
            chunks = [(hf, hh, c) for hf in range(2) for hh in range(2)
                      for c in range(3)]
            x1s = {}

            def load_chunk(ch):
                hf, hh, c = ch
                x1h = P.tile([128, HQ], mt.float32, tag="x1", bufs=3,
                             name=f"x1_{hf}_{hh}_{c}")
                nc.sync.dma_start(x1h[:], xv[:, hf, c, hh, :])
                x1s[ch] = x1h

            for ch in chunks[:4]:
                load_chunk(ch)

            # constants (issued after the first x loads; needed much later)
            wdr = P.tile([96, 5, 2, 128], mt.float8e4)
            nc.sync.dma_start(
                wdr[:], wdr_d.ap().rearrange("p (d t m) -> p d t m", d=5, t=2))
            cst = P.tile([128, 4], mt.float32)
            nc.sync.dma_start(cst[:], cst_d.ap())
            coef = P.tile([128, 8], mt.float32)
            if general_affine:
                nc.sync.dma_start(coef[:], coef_d.ap())

            for ci, (hf, hh, c) in enumerate(chunks):
                x1h = x1s[(hf, hh, c)]
                if general_affine:
                    nc.vector.tensor_scalar(
                        x1h[:], x1h[:], coef[:, c:c + 1],
                        coef[:, 3 + c:4 + c], AO.mult, AO.add)
                m1 = P.tile([128, HQ], mt.bfloat16, tag="m1", bufs=3,
                            name=f"m1_{hf}_{hh}_{c}")
                rint_eng = nc.vector if ci >= 10 else nc.gpsimd
                rint_eng.tensor_scalar(m1[:], x1h[:], MAGIC, MAGIC,
                                       AO.add, AO.subtract)
                nc.vector.tensor_tensor(m1[:], x1h[:], m1[:], AO.is_ge)
                xa1b = P.tile([128, 7, PW], mt.float8e4, tag="xa1b", bufs=2,
                              name=f"xa1b_{hf}_{hh}_{c}")
                if ci < 2:   # ring pads zeroed once; op3 writes interior only
                    nc.gpsimd.memset(xa1b[:, :, 0], 0.0)
                    nc.gpsimd.memset(xa1b[:, :, 225], 0.0)
                m1v = m1[:].rearrange("p (a b) -> p a b", a=7)
                if ci >= 10:
                    nc.vector.tensor_scalar(xa1b[:, :, 1:225],
                                            m1v[:, :, 1:225], 2.0, 1.0,
                                            AO.mult, AO.subtract)
                else:
                    nc.scalar.activation(xa1b[:, :, 1:225], m1v[:, :, 1:225],
                                         AF.Identity, bias=neg1[:],
                                         scale=two[:])
                # scatter all 8 segs in one DMA (partition p = 8g + s)
                abase = xa2f[16 * c:16 * c + 16, 0:1, 0:1]
                dst = AP(abase.tensor,
                         int(abase.offset) + (14 * hf + 7 * hh + 1) * PW,
                         [[int(abase.ap[0][0]), 16], [28 * PW, SEGS], [1, HQ]])
                nc.sync.dma_start(dst, xa1b[:].rearrange("p a b -> p (a b)"))
                # B-half scatter: B[r] = A[r+1], same source chunk
                bbase = xa2f[48 + 16 * c:64 + 16 * c, 0:1, 0:1]
                bdst = AP(bbase.tensor,
                          int(bbase.offset) + (14 * hf + 7 * hh) * PW,
                          [[int(bbase.ap[0][0]), 16], [28 * PW, SEGS], [1, HQ]])
                nc.sync.dma_start(bdst, xa1b[:].rearrange("p a b -> p (a b)"))
                if ci + 4 < len(chunks):
                    load_chunk(chunks[ci + 4])

            # ---- conv: DoubleRow matmuls ----
            xbase = xa2f[0:96, 0:1, 0:1]
            pstride = int(xbase.ap[0][0])
            xoff = int(xbase.offset)

            perf = mybir.MatmulPerfMode.DoubleRow
            for bp2 in range(NBANK // 2):
                ps = PS.tile([128, 2, 512], mt.float32, tag="ps", bufs=4,
                             name=f"ps_{bp2}")
                for half in range(2):
                    b = 2 * bp2 + half
                    y0 = 4 * b
                    for r in range(2):
                        for d, (t0, t1) in enumerate(DR_PAIRS):
                            o0 = (y0 + r + t0[0]) * PW + t0[1]
                            o1 = (y0 + r + t1[0]) * PW + t1[1]
                            mv = AP(xbase.tensor, xoff + o0,
                                    [[pstride, 96], [o1 - o0, 2], [1, 224]])
                            nc.tensor.matmul(
                                ps[:, half, 224 * r:224 * r + 224],
                                wdr[:, d, :, :], mv,
                                start=(d == 0), stop=(d == 4),
                                perf_mode=perf)
                nc.vector.tensor_scalar(y[:, 2 * bp2:2 * bp2 + 2, :],
                                        ps[:, :, 0:448],
                                        1.0, None, AO.mult, AO.add,
                                        accum_out=sums[:, bp2:bp2 + 1])
                nc.scalar.activation(ps[:, :, 0:448], ps[:, :, 0:448],
                                     AF.Square,
                                     accum_out=sqs[:, bp2:bp2 + 1])

            # ---- bypass loads (fp16, straight into y layout) ----
            byp_tiles = {}

            def load_byp(s):
                bp = P.tile([128, 7, 448], mt.float16, tag="byp", bufs=8,
                            name=f"byp_{s}")
                for ci in range(2):
                    src = AP(xhalf_d.ap().tensor, 6272 * s + 448 * ci,
                             [[H * W, COUT], [896, 7], [1, 448]])
                    nc.sync.dma_start(bp[64 * ci:64 * ci + 64, :, :], src)
                return bp

            for s in range(SEGS):
                byp_tiles[s] = load_byp(s)

            # ---- stats + collective + BN affine (all on 128 partitions) ----
            kc = P.tile([128, 2], mt.float32)
            sums2 = P.tile([128, 2], mt.float32)
            nc.vector.reduce_sum(sums2[:, 0:1], sums[:], axis=mybir.AxisListType.X)
            nc.vector.reduce_sum(sums2[:, 1:2], sqs[:], axis=mybir.AxisListType.X)
            cbin = D.tile([128, 2], mt.float32)
            cbout = D.tile([NCORES, 128, 2], mt.float32)
            nc.scalar.dma_start(cbin[:], sums2[:])
            nc.gpsimd.collective_compute(
                "AllGather", mybir.AluOpType.bypass,
                replica_groups=[list(range(NCORES))],
                ins=[cbin.opt()], outs=[cbout.opt()])
            # gather (core, half) entries onto BOTH partition halves
            gath = P.tile([128, 2, 2 * NCORES], mt.float32)
            cbt = cbout[:].rearrange("g (h p) q -> g h p q", h=2)
            for half in range(2):
                src = AP(cbt.tensor, 0,
                         [[2, 64], [1, 2], [128, 2 * NCORES]])
                nc.scalar.dma_start(gath[64 * half:64 * half + 64, :, :], src)
            mv2 = P.tile([128, 2], mt.float32)
            nc.vector.reduce_sum(mv2[:], gath[:], axis=mybir.AxisListType.X)
            nc.vector.tensor_scalar(mv2[:], mv2[:], 1.0 / float(B * H * W),
                                    None, AO.mult)

            m2t = P.tile([128, 1], mt.float32)
            nc.vector.tensor_tensor(m2t[:], mv2[:, 0:1], mv2[:, 0:1], AO.mult)
            vart = P.tile([128, 1], mt.float32)
            nc.vector.tensor_tensor(vart[:], mv2[:, 1:2], m2t[:], AO.subtract)
            t1 = P.tile([128, 1], mt.float32)
            nc.vector.tensor_tensor(t1[:], vart[:], cst[:, 0:1], AO.mult)
            nc.vector.tensor_scalar(t1[:], t1[:], BN_EPS, None, AO.add)
            sq = P.tile([128, 1], mt.float32)
            nc.scalar.activation(sq[:], t1[:], AF.Sqrt)
            rc = P.tile([128, 1], mt.float32)
            nc.vector.reciprocal(rc[:], sq[:])
            nc.vector.tensor_tensor(kc[:, 0:1], rc[:], cst[:, 1:2], AO.mult)
            mk = P.tile([128, 1], mt.float32)
            nc.vector.tensor_tensor(mk[:], mv2[:, 0:1], kc[:, 0:1], AO.mult)
            nc.vector.tensor_tensor(kc[:, 1:2], cst[:, 2:3], mk[:],
                                    AO.subtract)

            # ---- pass 2: affine + bypass + store ----
            for s in range(SEGS):
                bp = byp_tiles.pop(s)
                nc.scalar.activation(bp[:], bp[:], AF.Identity,
                                     bias=kc[:, 1:2])
                for (j0, nj) in ((0, 4), (4, 3)):
                    ob = P.tile([128, 4, 448], mt.float32, tag="ob", bufs=3,
                                name=f"ob_{s}_{j0}")
                    nc.vector.scalar_tensor_tensor(
                        ob[:, 0:nj, :], y[:, 7 * s + j0:7 * s + j0 + nj, :],
                        kc[:, 0:1], bp[:, j0:j0 + nj, :], AO.mult, AO.add)
                    nc.gpsimd.dma_start(
                        out_d.ap()[:, :, 7 * s + j0:7 * s + j0 + nj, :],
                        ob[:, 0:nj, :])

    nc.compile()
    return nc


def _get_nc(general_affine):
    key = ("nc", general_affine, NCORES)
    if key not in _cache:
        _cache[key] = _build(general_affine)
    return _cache[key]


def _pack_weights(wt):
    """wt [64, 48, 3, 3] (+-1 * A, slot-permuted) -> [96, 5, 2, 128] f32."""
    w = np.zeros((96, 5, 2, 128), np.float32)
    covered = set()
    for d, pair in enumerate(DR_PAIRS):
        for t, tl in enumerate(pair):
            if len(tl) == 3:
                continue
            delta, kw = tl
            for stack in (0, 1):
                for half, rho in ((0, 0), (1, 2)):
                    kh = delta + stack - rho
                    if 0 <= kh <= 2 and (rho, kh, kw) not in covered:
                        covered.add((rho, kh, kw))
                        w[48 * stack:48 * stack + 48, d, t,
                          64 * half:64 * half + 64] = wt[:, :, kh, kw].T
    assert len(covered) == 18
    return w


def _host_prep(alpha, epsilon, tau, A, weight, gamma, beta):
    import ml_dtypes
    f8 = ml_dtypes.float8_e4m3

    eps_v = np.asarray(epsilon, np.float32).reshape(-1)
    tau_v = np.asarray(tau, np.float32).reshape(-1)
    A_v = np.asarray(A, np.float32).reshape(-1)
    if eps_v.size == 1:
        eps_v = np.full(CIN, eps_v[0], np.float32)
    if tau_v.size == 1:
        tau_v = np.full(CIN, tau_v[0], np.float32)
    if A_v.size == 1:
        A_v = np.full(CIN, A_v[0], np.float32)

    general = not (np.all(eps_v == 0.0) and np.all(tau_v == 1.0))

    w = np.asarray(weight, np.float32)
    scale = np.mean(np.abs(w), axis=(1, 2, 3), dtype=np.float32)
    waff = np.sign(w) * A_v[None, :, None, None]
    wperm = waff[:, SLOT_TO_CH, :, :]
    wdr = _pack_weights(wperm).reshape(96, -1).astype(f8)

    cst = np.zeros((64, 4), np.float32)
    cst[:, 0] = scale * scale
    cst[:, 1] = np.asarray(gamma, np.float32).reshape(-1) * scale
    cst[:, 2] = np.asarray(beta, np.float32).reshape(-1)
    cst = np.tile(cst, (2, 1))

    coef = np.zeros((128, 8), np.float32)
    if general:
        for p in range(128):
            g = p // 8
            for c in range(3):
                ch = 45 + c if g == 15 else 15 * c + g
                coef[p, c] = 1.0 / tau_v[ch]
                coef[p, 3 + c] = -eps_v[ch] / tau_v[ch]
    return general, wdr, cst, coef


def _make_xdev(xi):
    """xi [48, 224, 224] f32 -> [128, 2, 3, 3164] (rows padded to 226)."""
    xp = np.zeros((CIN, H, PW), np.float32)
    xp[:, :, 1:225] = xi
    xr = xp.reshape(CIN, SEGS, 2, RQ)       # [ch, seg, hf, 14*226]
    p = np.arange(128)
    g_idx, s_idx = p // 8, p % 8
    out = np.empty((128, 2, 3, RQ), np.float32)
    for c in range(3):
        ch = np.where(g_idx == 15, 45 + c, 15 * c + g_idx)
        out[:, :, c, :] = xr[ch, s_idx, :, :]
    return out


def _make_xhalf(xi):
    """xi [48, 224, 224] f32 -> [64, H*W] fp16 (identity + 16 group means)."""
    xh = np.empty((COUT, H * W), np.float16)
    xh[0:CIN] = xi.reshape(CIN, -1).astype(np.float16)
    xf = xi.reshape(CIN, -1)
    xh[48:63] = xf[0:45].reshape(3, 15, -1).mean(axis=0,
                                                 dtype=np.float32).astype(np.float16)
    xh[63] = xf[45:48].mean(axis=0, dtype=np.float32).astype(np.float16)
    return xh


def kernel(x, alpha, epsilon, tau, A, weight, gamma, beta):
    from concourse import bass_utils

    x = np.asarray(x, np.float32)
    general, wdr, cst, coef = _host_prep(alpha, epsilon, tau, A,
                                         weight, gamma, beta)
    nc = _get_nc(general)

    in_maps = []
    for i in range(NCORES):
        xi = np.ascontiguousarray(x[i])
        in_maps.append({
            "xdev": _make_xdev(xi),
            "xhalf": _make_xhalf(xi),
            "wdr": wdr, "cst": cst, "coef": coef,
        })
    res = bass_utils.run_bass_kernel_spmd(nc, in_maps,
                                          core_ids=list(range(NCORES)))
    out = np.stack([
        res.results[i]["out"].reshape(2, COUT, NBANK, 2, 224)
        .transpose(1, 2, 0, 3, 4).reshape(COUT, H, W)
        for i in range(NCORES)
    ])
    return out.astype(np.float32)


# revision 47
# speedup vs baseline: 2.1877x; 1.0145x over previous
"""Trainium2 Bass kernel for nn_BiDenseConv2d (binarized 3x3 conv + sync-BN + channel bypass).

Shapes (hardcoded): x [8, 48, 224, 224] f32 -> out [8, 64, 224, 224] f32.
Sharding: data-parallel over batch, 1 image per core; BN stats all-reduced
([64,2] f32 AllGather); weights replicated.

Per-core pipeline (phases overlap via Tile dataflow):
  1. binarize (chunks of [128p=(seg,grp), 14 rows x 226]): rint via fp32 magic
     (Pool), is_ge (DVE), affine {1,0}->+-1 fp8 (ACT); column pads zeroed.
  2. scatter to conv layout xa2f [96, 226, 226] fp8 (A=rows, B=rows+1 copy),
     one 3164B-contiguous DMA per (c, hf, seg); B-half via 16 in-tile copies.
  3. conv: fp8 DoubleRow matmuls, M=128 (low 64 partitions = out rows y0+r,
     high = y0+2+r), N=224, 10 DR per 4-row bank; taps packed via tiles
     (delta in {0,2,4}) x kw over the 2-row stack; +-1 acts x +-1 weights give
     exact integer sums in PSUM f32, evicted to fp16.
  4. BN: sums (DVE evict accum) + sumsq (ACT Square accum); AllGather [64,2];
     k = gamma*s*rsqrt(s^2 var + eps), c = beta - mu*k  (s = mean|w|).
  5. bypass: host supplies xhalf fp16 [64, H*W] (48 identity channels + 16
     3-channel means); loaded per seg straight into the y layout; pass 2:
     ob = affine(y) (ACT) + byp (DVE), stored via Pool SWDGE.

Conv input channel at slot 16c+g is channel 15c+g (g<15) / 45+c (g=15),
folded into the weights host-side. Output layout matches the baseline.
"""
import sys
import numpy as np

sys.path.insert(0, '/opt/trn_rl_repo')

B, CIN, COUT, H, W = 8, 48, 64, 224, 224
NCORES = 8
SEGS, SEGR = 8, 28
PW = 226
RQ = 14 * PW            # 3164 elems per (c, hf) chunk row-block
NBANK = 56
BN_EPS = 1e-5
MAGIC = 12582912.0

_cache = {}

SLOT_TO_CH = np.zeros(48, np.int64)
for _c in range(3):
    for _g in range(16):
        SLOT_TO_CH[16 * _c + _g] = (45 + _c) if _g == 15 else (15 * _c + _g)

# DR tile pairs (delta, kw); 'z' = zero-weight tile (arbitrary in-bounds read)
DR_PAIRS = [((0, 0), (2, 0)),
            ((0, 1), (2, 1)),
            ((0, 2), (4, 0)),
            ((2, 2), (4, 1)),
            ((1, 0, 'z'), (4, 2))]


def _build(general_affine: bool):
    from concourse import bacc, tile, mybir
    from concourse.ap import AP
    mt = mybir.dt
    AO = mybir.AluOpType
    AF = mybir.ActivationFunctionType

    nc = bacc.Bacc("TRN2", target_bir_lowering=False, debug=False,
                   num_devices=NCORES)

    xdev_d = nc.dram_tensor("xdev", [128, 2, 3, RQ], mt.float32,
                            kind="ExternalInput")
    xhalf_d = nc.dram_tensor("xhalf", [COUT, H * W], mt.float16,
                             kind="ExternalInput")
    wdr_d = nc.dram_tensor("wdr", [96, 5 * 2 * 128], mt.float8e4,
                           kind="ExternalInput")
    cst_d = nc.dram_tensor("cst", [128, 4], mt.float32, kind="ExternalInput")
    coef_d = nc.dram_tensor("coef", [128, 8], mt.float32, kind="ExternalInput")
    out_d = nc.dram_tensor("out", [2, COUT, NBANK, 448], mt.float32,
                           kind="ExternalOutput")

    with tile.TileContext(nc) as tc:
        with tc.tile_pool(name="main", bufs=1) as P, \
             tc.tile_pool(name="psum", bufs=8, space="PSUM") as PS, \
             tc.tile_pool(name="dram", bufs=1, space="DRAM") as D:

            # ---- persistent tiles ----
            xa2f = P.tile([96, PW, PW], mt.float8e4)
            y = P.tile([128, NBANK, 448], mt.float16)
            sums = P.tile([128, NBANK // 2], mt.float32)
            sqs = P.tile([128, NBANK // 2], mt.float32)

            neg1 = P.tile([128, 1], mt.float32)
            nc.vector.memset(neg1[:], -1.0)
            scr1 = P.tile([128, 1], mt.float32)
            nc.vector.memset(scr1[:], 1.0)
            nc.scalar.activation(scr1[:], scr1[:], AF.Sqrt)
            two = P.tile([128, 1], mt.float32)
            nc.vector.memset(two[:], 2.0)

            # top/bottom pads; A row 224 is rewritten by the (h1, s7) scatter
            nc.vector.memset(xa2f[0:96, 0, :], 0.0)
            nc.vector.memset(xa2f[0:96, 224:226, :], 0.0)

            # ---- binarize + scatter, half-chunks (hf, hh, c) of 7 rows ----
            # all loads issued first so the SP queue never head-blocks them
            HQ = RQ // 2            # 1582 = 7*226
            xv = xdev_d.ap().rearrange("p f c (h q) -> p f c h q", h=2)
            chunks = [(hf, hh, c) for hf in range(2) for hh in range(2)
                      for c in range(3)]
            x1s = {}

            def load_chunk(ch):
                hf, hh, c = ch
                x1h = P.tile([128, HQ], mt.float32, tag="x1", bufs=3,
                             name=f"x1_{hf}_{hh}_{c}")
                nc.sync.dma_start(x1h[:], xv[:, hf, c, hh, :])
                x1s[ch] = x1h

            for ch in chunks[:4]:
                load_chunk(ch)

            # constants (issued after the first x loads; needed much later)
            wdr = P.tile([96, 5, 2, 128], mt.float8e4)
            nc.sync.dma_start(
                wdr[:], wdr_d.ap().rearrange("p (d t m) -> p d t m", d=5, t=2))
            cst = P.tile([128, 4], mt.float32)
            nc.sync.dma_start(cst[:], cst_d.ap())
            coef = P.tile([128, 8], mt.float32)
            if general_affine:
                nc.sync.dma_start(coef[:], coef_d.ap())

            for ci, (hf, hh, c) in enumerate(chunks):
                x1h = x1s[(hf, hh, c)]
                if general_affine:
                    nc.vector.tensor_scalar(
                        x1h[:], x1h[:], coef[:, c:c + 1],
                        coef[:, 3 + c:4 + c], AO.mult, AO.add)
                m1 = P.tile([128, HQ], mt.bfloat16, tag="m1", bufs=3,
                            name=f"m1_{hf}_{hh}_{c}")
                rint_eng = nc.vector if ci >= 10 else nc.gpsimd
                rint_eng.tensor_scalar(m1[:], x1h[:], MAGIC, MAGIC,
                                       AO.add, AO.subtract)
                nc.vector.tensor_tensor(m1[:], x1h[:], m1[:], AO.is_ge)
                xa1b = P.tile([128, 7, PW], mt.float8e4, tag="xa1b", bufs=2,
                              name=f"xa1b_{hf}_{hh}_{c}")
                if ci < 2:   # ring pads zeroed once; op3 writes interior only
                    nc.gpsimd.memset(xa1b[:, :, 0], 0.0)
                    nc.gpsimd.memset(xa1b[:, :, 225], 0.0)
                m1v = m1[:].rearrange("p (a b) -> p a b", a=7)
                if ci >= 10:
                    nc.vector.tensor_scalar(xa1b[:, :, 1:225],
                                            m1v[:, :, 1:225], 2.0, 1.0,
                                            AO.mult, AO.subtract)
                else:
                    nc.scalar.activation(xa1b[:, :, 1:225], m1v[:, :, 1:225],
                                         AF.Identity, bias=neg1[:],
                                         scale=two[:])
                # scatter all 8 segs in one DMA (partition p = 8g + s)
                abase = xa2f[16 * c:16 * c + 16, 0:1, 0:1]
                dst = AP(abase.tensor,
                         int(abase.offset) + (14 * hf + 7 * hh + 1) * PW,
                         [[int(abase.ap[0][0]), 16], [28 * PW, SEGS], [1, HQ]])
                nc.sync.dma_start(dst, xa1b[:].rearrange("p a b -> p (a b)"))
                # B-half scatter: B[r] = A[r+1], same source chunk
                bbase = xa2f[48 + 16 * c:64 + 16 * c, 0:1, 0:1]
                bdst = AP(bbase.tensor,
                          int(bbase.offset) + (14 * hf + 7 * hh) * PW,
                          [[int(bbase.ap[0][0]), 16], [28 * PW, SEGS], [1, HQ]])
                nc.sync.dma_start(bdst, xa1b[:].rearrange("p a b -> p (a b)"))
                if ci + 4 < len(chunks):
                    load_chunk(chunks[ci + 4])

            # ---- conv: DoubleRow matmuls ----
            xbase = xa2f[0:96, 0:1, 0:1]
            pstride = int(xbase.ap[0][0])
            xoff = int(xbase.offset)

            perf = mybir.MatmulPerfMode.DoubleRow
            for bp2 in range(NBANK // 2):
                ps = PS.tile([128, 2, 512], mt.float32, tag="ps", bufs=4,
                             name=f"ps_{bp2}")
                for half in range(2):
                    b = 2 * bp2 + half
                    y0 = 4 * b
                    for r in range(2):
                        for d, (t0, t1) in enumerate(DR_PAIRS):
                            o0 = (y0 + r + t0[0]) * PW + t0[1]
                            o1 = (y0 + r + t1[0]) * PW + t1[1]
                            mv = AP(xbase.tensor, xoff + o0,
                                    [[pstride, 96], [o1 - o0, 2], [1, 224]])
                            nc.tensor.matmul(
                                ps[:, half, 224 * r:224 * r + 224],
                                wdr[:, d, :, :], mv,
                                start=(d == 0), stop=(d == 4),
                                perf_mode=perf)
                nc.vector.tensor_scalar(y[:, 2 * bp2:2 * bp2 + 2, :],
                                        ps[:, :, 0:448],
                                        1.0, None, AO.mult, AO.add,
                                        accum_out=sums[:, bp2:bp2 + 1])
                nc.scalar.activation(ps[:, :, 0:448], ps[:, :, 0:448],
                                     AF.Square,
                                     accum_out=sqs[:, bp2:bp2 + 1])

            # ---- bypass loads (fp16, straight into y layout) ----
            byp_tiles = {}

            def load_byp(s):
                bp = P.tile([128, 7, 448], mt.float16, tag="byp", bufs=8,
                            name=f"byp_{s}")
                for ci in range(2):
                    src = AP(xhalf_d.ap().tensor, 6272 * s + 448 * ci,
                             [[H * W, COUT], [896, 7], [1, 448]])
                    nc.sync.dma_start(bp[64 * ci:64 * ci + 64, :, :], src)
                return bp

            for s in range(SEGS):
                byp_tiles[s] = load_byp(s)

            # ---- stats + collective + BN affine (all on 128 partitions) ----
            kc = P.tile([128, 2], mt.float32)
            sums2 = P.tile([128, 2], mt.float32)
            nc.vector.reduce_sum(sums2[:, 0:1], sums[:], axis=mybir.AxisListType.X)
            nc.vector.reduce_sum(sums2[:, 1:2], sqs[:], axis=mybir.AxisListType.X)
            cbin = D.tile([128, 2], mt.float32)
            cbout = D.tile([NCORES, 128, 2], mt.float32)
            nc.scalar.dma_start(cbin[:], sums2[:])
            nc.gpsimd.collective_compute(
                "AllGather", mybir.AluOpType.bypass,
                replica_groups=[list(range(NCORES))],
                ins=[cbin.opt()], outs=[cbout.opt()])
            # gather (core, half) entries onto BOTH partition halves
            gath = P.tile([128, 2, 2 * NCORES], mt.float32)
            cbt = cbout[:].rearrange("g (h p) q -> g h p q", h=2)
            for half in range(2):
                src = AP(cbt.tensor, 0,
                         [[2, 64], [1, 2], [128, 2 * NCORES]])
                nc.sync.dma_start(gath[64 * half:64 * half + 64, :, :], src)
            mv2 = P.tile([128, 2], mt.float32)
            nc.vector.reduce_sum(mv2[:], gath[:], axis=mybir.AxisListType.X)
            nc.vector.tensor_scalar(mv2[:], mv2[:], 1.0 / float(B * H * W),
                                    None, AO.mult)

            m2t = P.tile([128, 1], mt.float32)
            nc.vector.tensor_tensor(m2t[:], mv2[:, 0:1], mv2[:, 0:1], AO.mult)
            vart = P.tile([128, 1], mt.float32)
            nc.vector.tensor_tensor(vart[:], mv2[:, 1:2], m2t[:], AO.subtract)
            t1 = P.tile([128, 1], mt.float32)
            nc.vector.tensor_tensor(t1[:], vart[:], cst[:, 0:1], AO.mult)
            nc.vector.tensor_scalar(t1[:], t1[:], BN_EPS, None, AO.add)
            sq = P.tile([128, 1], mt.float32)
            nc.scalar.activation(sq[:], t1[:], AF.Sqrt)
            rc = P.tile([128, 1], mt.float32)
            nc.vector.reciprocal(rc[:], sq[:])
            nc.vector.tensor_tensor(kc[:, 0:1], rc[:], cst[:, 1:2], AO.mult)
            mk = P.tile([128, 1], mt.float32)
            nc.vector.tensor_tensor(mk[:], mv2[:, 0:1], kc[:, 0:1], AO.mult)
            nc.vector.tensor_tensor(kc[:, 1:2], cst[:, 2:3], mk[:],
                                    AO.subtract)

            # ---- pass 2: affine + bypass + store ----
            for s in range(SEGS):
                bp = byp_tiles.pop(s)
                if s == 0:
                    nc.vector.tensor_scalar(bp[:], bp[:], kc[:, 1:2], None,
                                            AO.add)
                else:
                    nc.scalar.activation(bp[:], bp[:], AF.Identity,
                                         bias=kc[:, 1:2])
                for (j0, nj) in ((0, 4), (4, 3)):
                    ob = P.tile([128, 4, 448], mt.float32, tag="ob", bufs=3,
                                name=f"ob_{s}_{j0}")
                    nc.vector.scalar_tensor_tensor(
                        ob[:, 0:nj, :], y[:, 7 * s + j0:7 * s + j0 + nj, :],
                        kc[:, 0:1], bp[:, j0:j0 + nj, :], AO.mult, AO.add)
                    nc.gpsimd.dma_start(
                        out_d.ap()[:, :, 7 * s + j0:7 * s + j0 + nj, :],
                        ob[:, 0:nj, :])

    nc.compile()
    return nc


def _get_nc(general_affine):
    key = ("nc", general_affine, NCORES)
    if key not in _cache:
        _cache[key] = _build(general_affine)
    return _cache[key]


def _pack_weights(wt):
    """wt [64, 48, 3, 3] (+-1 * A, slot-permuted) -> [96, 5, 2, 128] f32."""
    w = np.zeros((96, 5, 2, 128), np.float32)
    covered = set()
    for d, pair in enumerate(DR_PAIRS):
        for t, tl in enumerate(pair):
            if len(tl) == 3:
                continue
            delta, kw = tl
            for stack in (0, 1):
                for half, rho in ((0, 0), (1, 2)):
                    kh = delta + stack - rho
                    if 0 <= kh <= 2 and (rho, kh, kw) not in covered:
                        covered.add((rho, kh, kw))
                        w[48 * stack:48 * stack + 48, d, t,
                          64 * half:64 * half + 64] = wt[:, :, kh, kw].T
    assert len(covered) == 18
    return w


def _host_prep(alpha, epsilon, tau, A, weight, gamma, beta):
    import ml_dtypes
    f8 = ml_dtypes.float8_e4m3

    eps_v = np.asarray(epsilon, np.float32).reshape(-1)
    tau_v = np.asarray(tau, np.float32).reshape(-1)
    A_v = np.asarray(A, np.float32).reshape(-1)
    if eps_v.size == 1:
        eps_v = np.full(CIN, eps_v[0], np.float32)
    if tau_v.size == 1:
        tau_v = np.full(CIN, tau_v[0], np.float32)
    if A_v.size == 1:
        A_v = np.full(CIN, A_v[0], np.float32)

    general = not (np.all(eps_v == 0.0) and np.all(tau_v == 1.0))

    w = np.asarray(weight, np.float32)
    scale = np.mean(np.abs(w), axis=(1, 2, 3), dtype=np.float32)
    waff = np.sign(w) * A_v[None, :, None, None]
    wperm = waff[:, SLOT_TO_CH, :, :]
    wdr = _pack_weights(wperm).reshape(96, -1).astype(f8)

    cst = np.zeros((64, 4), np.float32)
    cst[:, 0] = scale * scale
    cst[:, 1] = np.asarray(gamma, np.float32).reshape(-1) * scale
    cst[:, 2] = np.asarray(beta, np.float32).reshape(-1)
    cst = np.tile(cst, (2, 1))

    coef = np.zeros((128, 8), np.float32)
    if general:
        for p in range(128):
            g = p // 8
            for c in range(3):
                ch = 45 + c if g == 15 else 15 * c + g
                coef[p, c] = 1.0 / tau_v[ch]
                coef[p, 3 + c] = -eps_v[ch] / tau_v[ch]
    return general, wdr, cst, coef


def _make_xdev(xi):
    """xi [48, 224, 224] f32 -> [128, 2, 3, 3164] (rows padded to 226)."""
    xp = np.zeros((CIN, H, PW), np.float32)
    xp[:, :, 1:225] = xi
    xr = xp.reshape(CIN, SEGS, 2, RQ)       # [ch, seg, hf, 14*226]
    p = np.arange(128)
    g_idx, s_idx = p // 8, p % 8
    out = np.empty((128, 2, 3, RQ), np.float32)
    for c in range(3):
        ch = np.where(g_idx == 15, 45 + c, 15 * c + g_idx)
        out[:, :, c, :] = xr[ch, s_idx, :, :]
    return out


def _make_xhalf(xi):
    """xi [48, 224, 224] f32 -> [64, H*W] fp16 (identity + 16 group means)."""
    xh = np.empty((COUT, H * W), np.float16)
    xh[0:CIN] = xi.reshape(CIN, -1).astype(np.float16)
    xf = xi.reshape(CIN, -1)
    xh[48:63] = xf[0:45].reshape(3, 15, -1).mean(axis=0,
                                                 dtype=np.float32).astype(np.float16)
    xh[63] = xf[45:48].mean(axis=0, dtype=np.float32).astype(np.float16)
    return xh


def kernel(x, alpha, epsilon, tau, A, weight, gamma, beta):
    from concourse import bass_utils

    x = np.asarray(x, np.float32)
    general, wdr, cst, coef = _host_prep(alpha, epsilon, tau, A,
                                         weight, gamma, beta)
    nc = _get_nc(general)

    in_maps = []
    for i in range(NCORES):
        xi = np.ascontiguousarray(x[i])
        in_maps.append({
            "xdev": _make_xdev(xi),
            "xhalf": _make_xhalf(xi),
            "wdr": wdr, "cst": cst, "coef": coef,
        })
    res = bass_utils.run_bass_kernel_spmd(nc, in_maps,
                                          core_ids=list(range(NCORES)))
    out = np.stack([
        res.results[i]["out"].reshape(2, COUT, NBANK, 2, 224)
        .transpose(1, 2, 0, 3, 4).reshape(COUT, H, W)
        for i in range(NCORES)
    ])
    return out.astype(np.float32)
